# revision 11
# baseline (speedup 1.0000x reference)
"""Trainium2 Bass kernel for a 4-layer dependency GNN (3x GraphConv + GAT).

Full inputs in, full output out. Internally nodes are sharded across 8
NeuronCores by dst ownership (1250 nodes/core, padded to 1280 = 10 blocks
of 128 per shard).

Design (v2 — dense-adjacency):
  - The per-core adjacency A_c [10000 src x 1250 dst] (entry = edge
    multiplicity) is built on the host from the integer edge list and
    uploaded as fp8 (0/1/2 are exact).  GraphConv aggregations run as dense
    matmuls: aggT[f, d] = sum_b h_blk[b]^T @ A_blk[b]  (lhsT = 128-row
    table block, rhs = fp8 A block streamed from DRAM).  No per-edge
    gather and no one-hot builds for any GraphConv layer.
  - GC1 is fully replicated: every core receives the full feature matrix
    (bf16) and computes the full h1 table locally -> no AllGather before
    the first aggregation.
  - GAT: the GAT output is only consumed through mean_h(out)@W3, which is
    linear, so W3 is folded through the attention: per edge only
    y_h = x3 @ (Wg_h @ W3) (4 scalars) and ex_h (4 scalars) are
    aggregated.  Per-node [el|y] rows (16 B) are computed densely and
    per-edge rows fetched with one dma_gather per dst tile; the dst-side
    er term is expanded with transposed one-hot matmuls.  One-hot matrices
    (0/1) are uploaded from the host in fp8 and streamed.
  - Per-layer cross-core tables move through 4 small AllGathers
    (320 KB x 3 + 2.5 KB).

Host-side work is limited to index manipulation (edge bucketing, one-hot /
adjacency construction from integer indices, wrapping/padding, integer
degree counts) and dtype casts; all floating-point model math runs on
device (degree^-1/2 included).
"""

import sys

import numpy as np

sys.path.insert(0, "/opt/trn_rl_repo")

import ml_dtypes  # noqa: E402

import concourse.bacc as bacc  # noqa: E402
import concourse.mybir as mybir  # noqa: E402
import concourse.tile as tile  # noqa: E402
from concourse import bass_utils  # noqa: E402
from concourse.masks import make_identity  # noqa: E402

N = 10000
E = 320000
IN_F = 256
HID = 128
HEADS = 4
NCORES = 8
NPC = N // NCORES           # nodes per core (1250)
P = 128
NTILES = (NPC + P - 1) // P  # dst tiles per core (10)
TILE_W = [min(P, NPC - t * P) for t in range(NTILES)]
NPAD = NTILES * P            # padded shard rows (1280)
NB_BLK = NCORES * NTILES     # global 128-row src blocks (80)
NFULL = NCORES * NPAD        # padded table rows (10240)
GB = 7                       # chunk batch in the GAT edge phase
NQ = 4                       # SWDGE queues

F32 = mybir.dt.float32
BF16 = mybir.dt.bfloat16
FP8 = mybir.dt.float8e4
I16 = mybir.dt.int16

NP_BF16 = ml_dtypes.bfloat16
NP_FP8 = ml_dtypes.float8_e4m3

_compiled_cache = {}


# ----------------------------------------------------------------------------
# host-side sharding / index preprocessing (integer work + dtype casts only)
# ----------------------------------------------------------------------------

def _wrap16(idx_block):
    """dma_gather index layout: [16, n/16] with [p, s] = idx[s*16+p],
    replicated across the 8 gpsimd cores (8 groups of 16 partitions)."""
    n = idx_block.shape[0]
    assert n % 16 == 0
    base = idx_block.reshape(n // 16, 16).T.astype(np.int16)
    return np.tile(base, (8, 1))


def _prow(n):
    """Row of global node n in the 10240-row padded table."""
    return NPAD * (n // NPC) + (n % NPC)


def _preprocess(src, dst):
    src = np.asarray(src).astype(np.int64).ravel()
    dst = np.asarray(dst).astype(np.int64).ravel()

    deg_out = np.bincount(src, minlength=N).astype(np.float32)
    deg_in = np.bincount(dst, minlength=N).astype(np.float32)

    # --- bucket edges by (dst core, dst tile) --------------------------------
    groups = {}
    counts = np.zeros((NCORES, NTILES), np.int64)
    for c in range(NCORES):
        sel = (dst // NPC) == c
        s_c = src[sel]
        d_c = dst[sel] - c * NPC
        order = np.argsort(d_c, kind="stable")
        s_c, d_c = s_c[order], d_c[order]
        t_c = d_c // P
        for t in range(NTILES):
            m = t_c == t
            groups[(c, t)] = (s_c[m], d_c[m] - t * P)
            counts[c, t] = int(m.sum())
    nchunks = int(-(-counts.max() // P))

    per_core = []
    for c in range(NCORES):
        # adjacency: [128, NB_BLK * NPC] fp8; block b covers global src rows
        # [NPC*(b//NTILES) + P*(b%NTILES), +128) (rows beyond the 98-wide
        # tail blocks stay zero).
        a = np.zeros((P, NB_BLK * NPC), np.float32)
        # one-hots for the GAT edge phase: [128, NTILES*nchunks*128]
        oh = np.zeros((P, NTILES * nchunks * P), np.float32)
        ohT = np.zeros((P, NTILES * nchunks * P), np.float32)
        idx_blocks = []
        for t in range(NTILES):
            s_g, dl_g = groups[(c, t)]
            ne = len(s_g)
            # adjacency entries for this tile's edges
            sl = s_g % NPC
            blk = NTILES * (s_g // NPC) + sl // P
            srow = sl % P
            np.add.at(a, (srow, blk * NPC + t * P + dl_g), 1.0)
            # per-edge slots: edge i -> chunk i//128, lane i%128
            ch = np.arange(ne) // P
            lane = np.arange(ne) % P
            base = (t * nchunks + ch) * P
            oh[lane, base + dl_g] = 1.0
            ohT[dl_g, base + lane] = 1.0
            # gather indices (padded rows use index 0 -> finite garbage,
            # masked by zero one-hot columns)
            idx = np.zeros(nchunks * P, np.int64)
            idx[:ne] = _prow(s_g)
            idx_blocks.append(_wrap16(idx))
        per_core.append(
            dict(
                a8=a.astype(NP_FP8),
                oh8=oh.astype(NP_FP8),
                ohT8=ohT.astype(NP_FP8),
                src16=np.concatenate(idx_blocks, axis=1),
            )
        )

    # --- degree tensors (raw counts; device computes clip+rsqrt) -------------
    deg = []
    for c in range(NCORES):
        own_out = deg_out[c * NPC : (c + 1) * NPC]
        own_in = deg_in[c * NPC : (c + 1) * NPC]
        dout_blk = np.zeros((P, NB_BLK), np.float32)
        for b in range(NB_BLK):
            g0 = NPC * (b // NTILES) + P * (b % NTILES)
            w = min(P, NPC * (b // NTILES) + NPC - g0)
            dout_blk[:w, b] = deg_out[g0 : g0 + w]
        deg.append(
            dict(
                doutblk=dout_blk,
                dinbc=np.tile(own_in[None, :], (P, 1)),
                doutownbc=np.tile(own_out[None, :], (P, 1)),
            )
        )
    return per_core, deg, nchunks


# ----------------------------------------------------------------------------
# device program
# ----------------------------------------------------------------------------

def _build(nchunks):
    IDXW = nchunks * 8           # int16 idx cols per tile
    NBATCH = -(-nchunks // GB)   # GAT chunk batches per tile
    AC = 10                      # adjacency blocks per DMA chunk
    GSPLIT = [(0, 512), (512, 1024), (1024, NPC)]  # dst column groups

    nc = bacc.Bacc(
        "TRN2", target_bir_lowering=False, debug=False, num_devices=NCORES,
        num_swdge_queues=NQ,
    )

    _q = [0]

    def next_q():
        _q[0] = (_q[0] + 1) % NQ
        return _q[0]

    # --- I/O ----------------------------------------------------------------
    xtw_in = nc.dram_tensor("xtw", [P, 2 * NFULL], BF16, kind="ExternalInput")
    a8_in = nc.dram_tensor("a8", [P, NB_BLK * NPC], FP8, kind="ExternalInput")
    oh8_in = nc.dram_tensor(
        "oh8", [P, NTILES * nchunks * P], FP8, kind="ExternalInput"
    )
    oht8_in = nc.dram_tensor(
        "oht8", [P, NTILES * nchunks * P], FP8, kind="ExternalInput"
    )
    src16_in = nc.dram_tensor("src16", [P, NTILES * IDXW], I16, kind="ExternalInput")
    w1w_in = nc.dram_tensor("w1w", [P, 2 * HID], BF16, kind="ExternalInput")
    w2_in = nc.dram_tensor("w2", [P, HID], BF16, kind="ExternalInput")
    wghT_in = nc.dram_tensor("wghT", [P, HEADS * HID], F32, kind="ExternalInput")
    alT_in = nc.dram_tensor("alT", [P, HEADS], F32, kind="ExternalInput")
    arT_in = nc.dram_tensor("arT", [P, HEADS], F32, kind="ExternalInput")
    w3c_in = nc.dram_tensor("w3c", [P, 1], F32, kind="ExternalInput")
    w3bc_in = nc.dram_tensor("w3bc", [P, HID], F32, kind="ExternalInput")
    bgbc_in = nc.dram_tensor("bgbc", [P, HEADS * HID], F32, kind="ExternalInput")
    b1c_in = nc.dram_tensor("b1c", [P, 1], F32, kind="ExternalInput")
    b2c_in = nc.dram_tensor("b2c", [P, 1], F32, kind="ExternalInput")
    b3c_in = nc.dram_tensor("b3c", [P, 1], F32, kind="ExternalInput")
    doutblk_in = nc.dram_tensor("doutblk", [P, NB_BLK], F32, kind="ExternalInput")
    dinbc_in = nc.dram_tensor("dinbc", [P, NPC], F32, kind="ExternalInput")
    doutownbc_in = nc.dram_tensor("doutownbc", [P, NPC], F32, kind="ExternalInput")
    risk_out = nc.dram_tensor("risk", [NPC, 1], F32, kind="ExternalOutput")

    rg = [list(range(NCORES))]

    with tile.TileContext(nc) as tc:
        with (
            tc.tile_pool(name="const", bufs=1) as cp,
            tc.tile_pool(name="tab", bufs=1) as tabp,
            tc.tile_pool(name="a8", bufs=2) as a8p,
            tc.tile_pool(name="xs", bufs=2) as xp,
            tc.tile_pool(name="oh", bufs=2) as ohp,
            tc.tile_pool(name="oht", bufs=2) as ohtp,
            tc.tile_pool(name="gel", bufs=3) as gelp,
            tc.tile_pool(name="work", bufs=3) as wp,
            tc.tile_pool(name="acc", bufs=1, space="PSUM") as pacc,
            tc.tile_pool(name="pmm", bufs=2, space="PSUM") as pw,
            tc.tile_pool(name="pga", bufs=1, space="PSUM") as pga,
            tc.tile_pool(name="psm", bufs=2, space="PSUM") as psm,
            tc.tile_pool(name="dram", bufs=1, space="DRAM") as dram,
        ):
            # --- DRAM interchange buffers ---------------------------------
            ag2_in = dram.tile([NPAD, HID], BF16)
            tab2_d = dram.tile([NFULL, HID], BF16)
            ag3_in = dram.tile([P, NPAD], BF16)
            tab3t_d = dram.tile([NCORES * P, NPAD], BF16)
            ely_d = dram.tile([NFULL, HID], BF16)  # payload in cols 0:8
            ag4_in = dram.tile([NPAD, 1], BF16)
            tabs_d = dram.tile([NFULL, 1], BF16)

            # --- resident constants ---------------------------------------
            def cload(name, dram_t, shape, dt):
                t = cp.tile(shape, dt, tag=name)
                nc.sync.dma_start(out=t[:], in_=dram_t[:])
                return t

            w1w = cp.tile([P, 2, HID], BF16, tag="w1w")
            nc.sync.dma_start(
                out=w1w[:], in_=w1w_in[:].rearrange("p (k f) -> p k f", k=2)
            )
            w2 = cload("w2", w2_in, [P, HID], BF16)
            wghT = cload("wghT", wghT_in, [P, HEADS * HID], F32)
            alT = cload("alT", alT_in, [P, HEADS], F32)
            arT = cload("arT", arT_in, [P, HEADS], F32)
            w3c = cload("w3c", w3c_in, [P, 1], F32)
            w3bc = cload("w3bc", w3bc_in, [P, HID], F32)
            bgbc = cload("bgbc", bgbc_in, [P, HEADS * HID], F32)
            b1c = cload("b1c", b1c_in, [P, 1], F32)
            b2c = cload("b2c", b2c_in, [P, 1], F32)
            b3c = cload("b3c", b3c_in, [P, 1], F32)
            src16 = cload("src16", src16_in, [P, NTILES * IDXW], I16)

            ident = cp.tile([P, P], F32)
            make_identity(nc, ident[:])

            zrow = cp.tile([P, HID], BF16, tag="zrow")
            nc.vector.memset(zrow[:], 0.0)

            def rsqrt_inplace(t):
                nc.vector.tensor_scalar(
                    out=t[:], in0=t[:], scalar1=1.0, scalar2=None,
                    op0=mybir.AluOpType.max,
                )
                nc.vector.reciprocal(out=t[:], in_=t[:])
                nc.scalar.activation(
                    out=t[:], in_=t[:], func=mybir.ActivationFunctionType.Sqrt
                )
                return t

            dso = rsqrt_inplace(cload("doutblk", doutblk_in, [P, NB_BLK], F32))
            ddbc = rsqrt_inplace(cload("dinbc", dinbc_in, [P, NPC], F32))
            dsbc = rsqrt_inplace(cload("doutownbc", doutownbc_in, [P, NPC], F32))

            # dsrc for own nodes as per-tile columns: dscol[p, t] =
            # dsrc[t*128+p], read off the broadcast dsbc rows via a diagonal
            # mask + free-dim reduce.
            dscol = cp.tile([P, NTILES], F32, tag="dscol")
            for t in range(NTILES):
                w = TILE_W[t]
                tmp = wp.tile([P, P], F32, tag="diag")
                nc.vector.tensor_tensor(
                    out=tmp[:w, :w], in0=dsbc[:w, t * P : t * P + w],
                    in1=ident[:w, :w], op=mybir.AluOpType.mult,
                )
                nc.vector.reduce_sum(
                    out=dscol[:w, t : t + 1], in_=tmp[:w, :w],
                    axis=mybir.AxisListType.X,
                )

            # bgW3 = (mean_h bg_h) @ W3 as a [128, 1] broadcast column
            bgm = cp.tile([P, HID], F32, tag="bgm")
            nc.vector.tensor_tensor(
                out=bgm[:], in0=bgbc[:, 0:HID], in1=bgbc[:, HID : 2 * HID],
                op=mybir.AluOpType.add,
            )
            nc.vector.tensor_tensor(
                out=bgm[:], in0=bgm[:], in1=bgbc[:, 2 * HID : 3 * HID],
                op=mybir.AluOpType.add,
            )
            nc.vector.tensor_tensor(
                out=bgm[:], in0=bgm[:], in1=bgbc[:, 3 * HID : 4 * HID],
                op=mybir.AluOpType.add,
            )
            nc.vector.tensor_scalar(
                out=bgm[:], in0=bgm[:], scalar1=0.25, scalar2=None,
                op0=mybir.AluOpType.mult,
            )
            bgw3 = cp.tile([P, 1], F32, tag="bgw3")
            nc.vector.tensor_tensor(
                out=bgm[:], in0=bgm[:], in1=w3bc[:], op=mybir.AluOpType.mult
            )
            nc.vector.reduce_sum(out=bgw3[:], in_=bgm[:], axis=mybir.AxisListType.X)

            # ALY [128, 8] = [AL | AY], AR [128, 4]: per head h,
            # col = Wg_h^T-matmul with [alT_h | arT_h | W3]
            alyr_t = psm.tile([P, 512], F32, tag="small", space="PSUM")
            alyr_ps = alyr_t[:, 0:3]
            aly = cp.tile([P, 2 * HEADS], BF16, tag="aly")
            ar4 = cp.tile([P, HEADS], BF16, tag="ar4")
            for h in range(HEADS):
                rhs3 = wp.tile([P, 3], F32, tag="rhs3")
                nc.vector.tensor_copy(out=rhs3[:, 0:1], in_=alT[:, h : h + 1])
                nc.vector.tensor_copy(out=rhs3[:, 1:2], in_=arT[:, h : h + 1])
                nc.vector.tensor_copy(out=rhs3[:, 2:3], in_=w3c[:, 0:1])
                nc.tensor.matmul(
                    out=alyr_ps, lhsT=wghT[:, h * HID : (h + 1) * HID],
                    rhs=rhs3[:], start=True, stop=True,
                )
                nc.vector.tensor_copy(out=aly[:, h : h + 1], in_=alyr_t[:, 0:1])
                nc.vector.tensor_copy(out=ar4[:, h : h + 1], in_=alyr_t[:, 1:2])
                nc.vector.tensor_copy(
                    out=aly[:, HEADS + h : HEADS + h + 1], in_=alyr_t[:, 2:3]
                )

            # zero the pad rows of the DRAM AG buffers once
            nc.sync.dma_start(out=ag2_in[NPC:NPAD, :], in_=zrow[: NPAD - NPC, :])
            nc.sync.dma_start(
                out=ag4_in[NPC:NPAD, :], in_=zrow[: NPAD - NPC, 0:1]
            )

            # =============== GC1: replicated node transform ================
            # h1_full[n, f] = dsrc[n] * (x[n] @ W1), all 10240 padded rows.
            h1 = tabp.tile([P, NB_BLK, HID], BF16, tag="tab")
            XC = 10  # blocks per xT stream chunk
            for cc0 in range(0, NB_BLK, XC):
                xs = xp.tile([P, XC, 2, P], BF16, tag="xs")
                nc.sync.dma_start(
                    out=xs[:],
                    in_=xtw_in[
                        :, cc0 * 2 * P : (cc0 + XC) * 2 * P
                    ].rearrange("p (b k q) -> p b k q", k=2, q=P),
                )
                for bi in range(XC):
                    b = cc0 + bi
                    ps = pw.tile([P, 512], F32, tag="mmw", space="PSUM")
                    nc.tensor.matmul(
                        out=ps[:, 0:HID], lhsT=xs[:, bi, 0, :],
                        rhs=w1w[:, 0, :], start=True, stop=False,
                    )
                    nc.tensor.matmul(
                        out=ps[:, 0:HID], lhsT=xs[:, bi, 1, :],
                        rhs=w1w[:, 1, :], start=False, stop=True,
                    )
                    nc.vector.tensor_scalar(
                        out=h1[:, b, :], in0=ps[:, 0:HID],
                        scalar1=dso[:, b : b + 1],
                        scalar2=None, op0=mybir.AluOpType.mult,
                    )

            # =============== shared dense-aggregation pass =================
            def dense_agg(tab_sb):
                """aggT[f, d] accumulated over all 80 blocks; returns the
                3 PSUM group tiles (live until epilogue reads them)."""
                accs = [
                    pacc.tile(
                        [P, 512], F32, tag=f"acc{g}", space="PSUM",
                        name=f"acc{g}",
                    )
                    for g in range(3)
                ]
                for ac0 in range(0, NB_BLK, AC):
                    a8s = a8p.tile([P, AC, NPC], FP8, tag="a8s")
                    nc.sync.dma_start(
                        out=a8s[:],
                        in_=a8_in[:, ac0 * NPC : (ac0 + AC) * NPC].rearrange(
                            "p (b d) -> p b d", d=NPC
                        ),
                    )
                    for bi in range(AC):
                        b = ac0 + bi
                        for g, (g0, g1) in enumerate(GSPLIT):
                            nc.tensor.matmul(
                                out=accs[g][:, : g1 - g0],
                                lhsT=tab_sb[:, b, :],
                                rhs=a8s[:, bi, g0:g1],
                                start=(b == 0), stop=(b == NB_BLK - 1),
                                skip_group_check=True,
                            )
                return accs

            # =============== GC1 agg + GC2 node ===========================
            accs = dense_agg(h1)
            x2s = wp.tile([P, NPC], BF16, tag="x2s")
            for g, (g0, g1) in enumerate(GSPLIT):
                gw = g1 - g0
                t1 = wp.tile([P, 512], F32, tag="epi1")
                nc.vector.tensor_tensor(
                    out=t1[:, :gw], in0=accs[g][:, :gw], in1=ddbc[:, g0:g1],
                    op=mybir.AluOpType.mult,
                )
                nc.scalar.activation(
                    out=t1[:, :gw], in_=t1[:, :gw],
                    func=mybir.ActivationFunctionType.Relu,
                    bias=b1c[:, 0:1], scale=1.0,
                )
                nc.vector.tensor_tensor(
                    out=x2s[:, g0:g1], in0=t1[:, :gw], in1=dsbc[:, g0:g1],
                    op=mybir.AluOpType.mult,
                )
            # h2T[f2, d] = W2^T @ x2s
            h2t = wp.tile([P, NPC], F32, tag="h2t")
            for g, (g0, g1) in enumerate(GSPLIT):
                ps = pw.tile([P, 512], F32, tag="mmw", space="PSUM")
                nc.tensor.matmul(
                    out=ps[:, : g1 - g0], lhsT=w2[:], rhs=x2s[:, g0:g1],
                    start=True, stop=True,
                )
                nc.vector.tensor_copy(out=h2t[:, g0:g1], in_=ps[:, : g1 - g0])
            # transpose h2T tiles -> row-major h2 shard -> AG2 buffer
            for t in range(NTILES):
                w = TILE_W[t]
                pt = pw.tile([P, 512], F32, tag="mmw", space="PSUM")
                nc.tensor.transpose(
                    out=pt[:w, 0:P], in_=h2t[:, t * P : t * P + w],
                    identity=ident[:],
                )
                h2r = wp.tile([P, HID], BF16, tag="h2r")
                nc.vector.tensor_copy(out=h2r[:w, :], in_=pt[:w, 0:P])
                nc.sync.dma_start(
                    out=ag2_in[t * P : t * P + w, :], in_=h2r[:w, :]
                )

            nc.gpsimd.collective_compute(
                "AllGather", mybir.AluOpType.bypass, replica_groups=rg,
                ins=[ag2_in[:].opt()], outs=[tab2_d[:].opt()],
            )

            # =============== GC2 agg + GAT node prep ======================
            tab2 = tabp.tile([P, NB_BLK, HID], BF16, tag="tab")
            nc.sync.dma_start(
                out=tab2[:], in_=tab2_d[:].rearrange("(b p) f -> p b f", p=P)
            )
            accs = dense_agg(tab2)
            # x3T [f, d] bf16 (padded cols zeroed for the AG)
            x3t = cp.tile([P, NPAD], BF16, tag="x3t")
            nc.vector.memset(x3t[:, NPC:NPAD], 0.0)
            for g, (g0, g1) in enumerate(GSPLIT):
                gw = g1 - g0
                t1 = wp.tile([P, 512], F32, tag="epi1")
                nc.vector.tensor_tensor(
                    out=t1[:, :gw], in0=accs[g][:, :gw], in1=ddbc[:, g0:g1],
                    op=mybir.AluOpType.mult,
                )
                nc.scalar.activation(
                    out=x3t[:, g0:g1], in_=t1[:, :gw],
                    func=mybir.ActivationFunctionType.Relu,
                    bias=b2c[:, 0:1], scale=1.0,
                )
            nc.sync.dma_start(out=ag3_in[:], in_=x3t[:])
            # er per dst tile: [d, 4] = x3T_tile^T @ AR
            er_sb = cp.tile([P, NTILES * HEADS], BF16, tag="er_sb")
            nc.vector.memset(er_sb[:], 0.0)
            for t in range(NTILES):
                w = TILE_W[t]
                ps = psm.tile([P, 512], F32, tag="small", space="PSUM")
                nc.tensor.matmul(
                    out=ps[:w, 0:HEADS], lhsT=x3t[:, t * P : t * P + w],
                    rhs=ar4[:], start=True, stop=True,
                )
                nc.vector.tensor_copy(
                    out=er_sb[:w, t * HEADS : (t + 1) * HEADS],
                    in_=ps[:w, 0:HEADS],
                )

            nc.gpsimd.collective_compute(
                "AllGather", mybir.AluOpType.bypass, replica_groups=rg,
                ins=[ag3_in[:].opt()], outs=[tab3t_d[:].opt()],
            )

            # =============== ely table: [el|y] per node ===================
            tab3t = tabp.tile([P, NCORES, NTILES, P], BF16, tag="tab")
            nc.sync.dma_start(
                out=tab3t[:],
                in_=tab3t_d[:].rearrange(
                    "(c p) (j q) -> p c j q", p=P, q=P
                ),
            )
            ely = cp.tile([P, NB_BLK, 8], BF16, tag="ely")
            for b in range(NB_BLK):
                ps = psm.tile([P, 512], F32, tag="small", space="PSUM")
                nc.tensor.matmul(
                    out=ps[:, 0:8], lhsT=tab3t[:, b // NTILES, b % NTILES, :],
                    rhs=aly[:], start=True, stop=True,
                )
                nc.vector.tensor_copy(out=ely[:, b, :], in_=ps[:, 0:8])
            nc.sync.dma_start(
                out=ely_d[:, 0:8].rearrange("(b p) f -> p b f", p=P),
                in_=ely[:],
            )

            # =============== GAT edge phase ===============================
            for t in range(NTILES):
                w = TILE_W[t]
                gel = gelp.tile([P, nchunks, HID], BF16, tag="gel")
                nc.gpsimd.dma_gather(
                    gel[:], ely_d[:], src16[:, t * IDXW : (t + 1) * IDXW],
                    nchunks * P, nchunks * P, HID, elem_step=HID,
                    single_packet=False, queue_num=next_q(),
                )
                oh8 = ohp.tile([P, nchunks, P], FP8, tag="oh8")
                nc.sync.dma_start(
                    out=oh8[:],
                    in_=oh8_in[
                        :, t * nchunks * P : (t + 1) * nchunks * P
                    ].rearrange("p (c d) -> p c d", d=P),
                )
                oht8 = ohtp.tile([P, nchunks, P], FP8, tag="oht8")
                nc.sync.dma_start(
                    out=oht8[:],
                    in_=oht8_in[
                        :, t * nchunks * P : (t + 1) * nchunks * P
                    ].rearrange("p (c e) -> p c e", e=P),
                )
                acc_t = pga.tile([P, 512], F32, tag="gacc", space="PSUM")
                acc = acc_t[:, 0:8]
                ert = er_sb[:, t * HEADS : (t + 1) * HEADS]
                for b0 in range(0, nchunks, GB):
                    b1 = min(b0 + GB, nchunks)
                    nb = b1 - b0
                    erp_t = psm.tile([P, 512], F32, tag="small", space="PSUM")
                    erp = erp_t[:, 0 : GB * HEADS].rearrange(
                        "p (c h) -> p c h", h=HEADS
                    )
                    for cc in range(b0, b1):
                        nc.tensor.matmul(
                            out=erp[:, cc - b0, :], lhsT=oht8[:, cc, :],
                            rhs=ert, start=True, stop=True,
                            skip_group_check=True,
                        )
                    # e = lrelu(el + er); ex = exp(e)
                    e_all = wp.tile([P, GB, HEADS], F32, tag="e_all")
                    nc.vector.tensor_tensor(
                        out=e_all[:, :nb, :], in0=gel[:, b0:b1, 0:HEADS],
                        in1=erp[:, :nb, :], op=mybir.AluOpType.add,
                    )
                    nc.vector.scalar_tensor_tensor(
                        out=e_all[:, :nb, :], in0=e_all[:, :nb, :], scalar=0.2,
                        in1=e_all[:, :nb, :], op0=mybir.AluOpType.mult,
                        op1=mybir.AluOpType.max,
                    )
                    ex_all = wp.tile([P, GB, HEADS], F32, tag="ex_all")
                    nc.scalar.activation(
                        out=ex_all[:, :nb, :], in_=e_all[:, :nb, :],
                        func=mybir.ActivationFunctionType.Exp,
                    )
                    # rhs = [y*ex | ex] bf16
                    rp = wp.tile([P, GB, 8], BF16, tag="rp")
                    nc.vector.tensor_tensor(
                        out=rp[:, :nb, 0:HEADS], in0=gel[:, b0:b1, HEADS:8],
                        in1=ex_all[:, :nb, :], op=mybir.AluOpType.mult,
                    )
                    nc.vector.tensor_copy(
                        out=rp[:, :nb, HEADS:8], in_=ex_all[:, :nb, :]
                    )
                    for cc in range(b0, b1):
                        nc.tensor.matmul(
                            out=acc, lhsT=oh8[:, cc, :],
                            rhs=rp[:, cc - b0, :],
                            start=(cc == 0), stop=(cc == nchunks - 1),
                            skip_group_check=True,
                        )
                # epilogue: s = dsrc * (mean_h(yagg/den) + bgW3)
                den = wp.tile([P, HEADS], F32, tag="den")
                nc.vector.tensor_scalar(
                    out=den[:], in0=acc_t[:, HEADS:8], scalar1=1e-30,
                    scalar2=None, op0=mybir.AluOpType.max,
                )
                nc.vector.reciprocal(out=den[:], in_=den[:])
                wy = wp.tile([P, HEADS], F32, tag="wy")
                nc.vector.tensor_tensor(
                    out=wy[:], in0=acc_t[:, 0:HEADS], in1=den[:],
                    op=mybir.AluOpType.mult,
                )
                sv = wp.tile([P, 1], F32, tag="sv")
                nc.vector.reduce_sum(out=sv[:], in_=wy[:], axis=mybir.AxisListType.X)
                nc.vector.scalar_tensor_tensor(
                    out=sv[:], in0=sv[:], scalar=0.25, in1=bgw3[:],
                    op0=mybir.AluOpType.mult, op1=mybir.AluOpType.add,
                )
                svb = wp.tile([P, 1], BF16, tag="svb")
                nc.vector.tensor_scalar(
                    out=svb[:], in0=sv[:], scalar1=dscol[:, t : t + 1],
                    scalar2=None, op0=mybir.AluOpType.mult,
                )
                nc.sync.dma_start(
                    out=ag4_in[t * P : t * P + w, :], in_=svb[:w, :]
                )

            nc.gpsimd.collective_compute(
                "AllGather", mybir.AluOpType.bypass, replica_groups=rg,
                ins=[ag4_in[:].opt()], outs=[tabs_d[:].opt()],
            )

            # =============== GC3: dense matvec + sigmoid ==================
            s_sb = cp.tile([P, NB_BLK], BF16, tag="s_sb")
            nc.sync.dma_start(
                out=s_sb[:], in_=tabs_d[:].rearrange("(b p) one -> p (b one)", p=P)
            )
            acc3 = [
                pacc.tile(
                    [P, 512], F32, tag=f"acc{g}", space="PSUM", name=f"acc3{g}"
                )
                for g in range(3)
            ]
            for ac0 in range(0, NB_BLK, AC):
                a8s = a8p.tile([P, AC, NPC], FP8, tag="a8s")
                nc.sync.dma_start(
                    out=a8s[:],
                    in_=a8_in[:, ac0 * NPC : (ac0 + AC) * NPC].rearrange(
                        "p (b d) -> p b d", d=NPC
                    ),
                )
                for bi in range(AC):
                    b = ac0 + bi
                    for g, (g0, g1) in enumerate(GSPLIT):
                        nc.tensor.matmul(
                            out=acc3[g][0:1, : g1 - g0],
                            lhsT=s_sb[:, b : b + 1],
                            rhs=a8s[:, bi, g0:g1],
                            start=(b == 0), stop=(b == NB_BLK - 1),
                            skip_group_check=True,
                        )
            risk_sb = wp.tile([P, NPC], F32, tag="risk")
            for g, (g0, g1) in enumerate(GSPLIT):
                gw = g1 - g0
                nc.vector.tensor_tensor(
                    out=risk_sb[0:1, g0:g1], in0=acc3[g][0:1, :gw],
                    in1=ddbc[0:1, g0:g1], op=mybir.AluOpType.mult,
                )
            nc.scalar.activation(
                out=risk_sb[0:1, :], in_=risk_sb[0:1, :],
                func=mybir.ActivationFunctionType.Sigmoid,
                bias=b3c[0:1, 0:1], scale=1.0,
            )
            nc.sync.dma_start(
                out=risk_out[:].rearrange("d one -> one d"),
                in_=risk_sb[0:1, :],
            )

    nc.compile()
    return nc


# ----------------------------------------------------------------------------
# host driver
# ----------------------------------------------------------------------------

def _get_program(nchunks):
    if nchunks not in _compiled_cache:
        _compiled_cache[nchunks] = _build(nchunks)
    return _compiled_cache[nchunks]


def _install_ntff_hook():
    """Profiling support: register the NTFF hook bass_utils expects when this
    image's antenv package lacks axon_hooks. Best-effort, trace-path only."""
    import types

    try:
        import antenv.axon_hooks  # noqa: F401

        return
    except ImportError:
        pass
    try:
        import antenv
        from trn_agent_boot.trn_boot import _ntff_profile_via_ctypes

        hook = _ntff_profile_via_ctypes("/opt/axon/libaxon_pjrt.so")
        mod = types.ModuleType("antenv.axon_hooks")
        mod.get_axon_ntff_profile_hook = lambda: hook
        mod.set_axon_ntff_profile_hook = lambda h: None
        sys.modules["antenv.axon_hooks"] = mod
        antenv.axon_hooks = mod
    except Exception:
        pass


def kernel(
    features, src, dst, W1, b1, W2, b2, W3, b3, Wg, attn_l, attn_r, bg,
    _trace=False,
):
    features = np.asarray(features, np.float32)
    per_core, deg, nchunks = _preprocess(src, dst)
    nc = _get_program(nchunks)

    # full features, wrapped + padded per 128-row block:
    # xtw[p, b, k, q] = x[row b*128+q, k*128+p]
    xpad = np.zeros((NFULL, IN_F), np.float32)
    for c in range(NCORES):
        xpad[c * NPAD : c * NPAD + NPC] = features[c * NPC : (c + 1) * NPC]
    xtw = (
        xpad.reshape(NB_BLK, P, 2, P)        # [b, q, k, p]
        .transpose(3, 0, 2, 1)               # [p, b, k, q]
        .reshape(P, 2 * NFULL)
        .astype(NP_BF16)
    )

    W1 = np.asarray(W1, np.float32)
    w1w = np.concatenate([W1[:P, :], W1[P:, :]], axis=1).astype(NP_BF16)
    Wg = np.asarray(Wg, np.float32)
    wghT = np.zeros((P, HEADS * HID), np.float32)
    for h in range(HEADS):
        wghT[:, h * HID : (h + 1) * HID] = Wg[:, h * HID : (h + 1) * HID].T

    common = dict(
        xtw=xtw,
        w1w=w1w,
        w2=np.asarray(W2, np.float32).astype(NP_BF16),
        wghT=wghT,
        alT=np.asarray(attn_l, np.float32).T.copy(),
        arT=np.asarray(attn_r, np.float32).T.copy(),
        w3c=np.asarray(W3, np.float32).reshape(P, 1),
        w3bc=np.tile(np.asarray(W3, np.float32).reshape(1, -1), (P, 1)),
        bgbc=np.tile(np.asarray(bg, np.float32).reshape(1, -1), (P, 1)),
        b1c=np.asarray(b1, np.float32).reshape(P, 1),
        b2c=np.asarray(b2, np.float32).reshape(P, 1),
        b3c=np.full((P, 1), np.float32(np.asarray(b3).reshape(-1)[0])),
    )
    in_maps = []
    for c in range(NCORES):
        m = dict(common)
        m["a8"] = per_core[c]["a8"]
        m["oh8"] = per_core[c]["oh8"]
        m["oht8"] = per_core[c]["ohT8"]
        m["src16"] = per_core[c]["src16"]
        m["doutblk"] = deg[c]["doutblk"]
        m["dinbc"] = deg[c]["dinbc"]
        m["doutownbc"] = deg[c]["doutownbc"]
        in_maps.append(m)

    if _trace:
        _install_ntff_hook()
    res = bass_utils.run_bass_kernel_spmd(
        nc, in_maps, core_ids=list(range(NCORES)), trace=_trace
    )
    out = np.concatenate([res.results[c]["risk"] for c in range(NCORES)], axis=0)
    if _trace:
        kernel.last_exec_time_ns = res.exec_time_ns
        kernel.last_results = res
    return out.astype(np.float32)


# revision 14
# speedup vs baseline: 1.0734x; 1.0734x over previous
"""Trainium2 Bass kernel for a 4-layer dependency GNN (3x GraphConv + GAT).

Full inputs in, full output out. Internally nodes are sharded across 8
NeuronCores by dst ownership (1250 nodes/core, padded to 1280 = 10 blocks
of 128 per shard).

Design (v2 — dense-adjacency):
  - The per-core adjacency A_c [10000 src x 1250 dst] (entry = edge
    multiplicity) is built on the host from the integer edge list and
    uploaded as fp8 (0/1/2 are exact).  GraphConv aggregations run as dense
    matmuls: aggT[f, d] = sum_b h_blk[b]^T @ A_blk[b]  (lhsT = 128-row
    table block, rhs = fp8 A block streamed from DRAM).  No per-edge
    gather and no one-hot builds for any GraphConv layer.
  - GC1 is fully replicated: every core receives the full feature matrix
    (bf16) and computes the full h1 table locally -> no AllGather before
    the first aggregation.
  - GAT: the GAT output is only consumed through mean_h(out)@W3, which is
    linear, so W3 is folded through the attention: per edge only
    y_h = x3 @ (Wg_h @ W3) (4 scalars) and ex_h (4 scalars) are
    aggregated.  Per-node [el|y] rows (16 B) are computed densely and
    per-edge rows fetched with one dma_gather per dst tile; the dst-side
    er term is expanded with transposed one-hot matmuls.  One-hot matrices
    (0/1) are uploaded from the host in fp8 and streamed.
  - Per-layer cross-core tables move through 4 small AllGathers
    (320 KB x 3 + 2.5 KB).

Host-side work is limited to index manipulation (edge bucketing, one-hot /
adjacency construction from integer indices, wrapping/padding, integer
degree counts) and dtype casts; all floating-point model math runs on
device (degree^-1/2 included).
"""

import sys

import numpy as np

sys.path.insert(0, "/opt/trn_rl_repo")

import ml_dtypes  # noqa: E402

import concourse.bacc as bacc  # noqa: E402
import concourse.mybir as mybir  # noqa: E402
import concourse.tile as tile  # noqa: E402
from concourse import bass_utils  # noqa: E402
from concourse.masks import make_identity  # noqa: E402

N = 10000
E = 320000
IN_F = 256
HID = 128
HEADS = 4
NCORES = 8
NPC = N // NCORES           # nodes per core (1250)
P = 128
NTILES = (NPC + P - 1) // P  # dst tiles per core (10)
TILE_W = [min(P, NPC - t * P) for t in range(NTILES)]
NPAD = NTILES * P            # padded shard rows (1280)
NB_BLK = NCORES * NTILES     # global 128-row src blocks (80)
NFULL = NCORES * NPAD        # padded table rows (10240)
GB = 7                       # chunk batch in the GAT edge phase
NQ = 4                       # SWDGE queues

F32 = mybir.dt.float32
BF16 = mybir.dt.bfloat16
FP8 = mybir.dt.float8e4
I16 = mybir.dt.int16

NP_BF16 = ml_dtypes.bfloat16
NP_FP8 = ml_dtypes.float8_e4m3

_compiled_cache = {}


# ----------------------------------------------------------------------------
# host-side sharding / index preprocessing (integer work + dtype casts only)
# ----------------------------------------------------------------------------

def _wrap16(idx_block):
    """dma_gather index layout: [16, n/16] with [p, s] = idx[s*16+p],
    replicated across the 8 gpsimd cores (8 groups of 16 partitions)."""
    n = idx_block.shape[0]
    assert n % 16 == 0
    base = idx_block.reshape(n // 16, 16).T.astype(np.int16)
    return np.tile(base, (8, 1))


def _prow(n):
    """Row of global node n in the 10240-row padded table."""
    return NPAD * (n // NPC) + (n % NPC)


def _preprocess(src, dst):
    src = np.asarray(src).astype(np.int64).ravel()
    dst = np.asarray(dst).astype(np.int64).ravel()

    deg_out = np.bincount(src, minlength=N).astype(np.float32)
    deg_in = np.bincount(dst, minlength=N).astype(np.float32)

    # --- bucket edges by (dst core, dst tile) --------------------------------
    groups = {}
    counts = np.zeros((NCORES, NTILES), np.int64)
    for c in range(NCORES):
        sel = (dst // NPC) == c
        s_c = src[sel]
        d_c = dst[sel] - c * NPC
        order = np.argsort(d_c, kind="stable")
        s_c, d_c = s_c[order], d_c[order]
        t_c = d_c // P
        for t in range(NTILES):
            m = t_c == t
            groups[(c, t)] = (s_c[m], d_c[m] - t * P)
            counts[c, t] = int(m.sum())
    nchunks = int(-(-counts.max() // P))

    per_core = []
    for c in range(NCORES):
        # adjacency: [128, NB_BLK * NPC] fp8; block b covers global src rows
        # [NPC*(b//NTILES) + P*(b%NTILES), +128) (rows beyond the 98-wide
        # tail blocks stay zero).
        a = np.zeros((P, NB_BLK * NPC), np.float32)
        # one-hots for the GAT edge phase: [128, NTILES*nchunks*128]
        oh = np.zeros((P, NTILES * nchunks * P), np.float32)
        ohT = np.zeros((P, NTILES * nchunks * P), np.float32)
        idx_blocks = []
        for t in range(NTILES):
            s_g, dl_g = groups[(c, t)]
            ne = len(s_g)
            # adjacency entries for this tile's edges
            sl = s_g % NPC
            blk = NTILES * (s_g // NPC) + sl // P
            srow = sl % P
            np.add.at(a, (srow, blk * NPC + t * P + dl_g), 1.0)
            # per-edge slots: edge i -> chunk i//128, lane i%128
            ch = np.arange(ne) // P
            lane = np.arange(ne) % P
            base = (t * nchunks + ch) * P
            oh[lane, base + dl_g] = 1.0
            ohT[dl_g, base + lane] = 1.0
            # gather indices (padded rows use index 0 -> finite garbage,
            # masked by zero one-hot columns)
            idx = np.zeros(nchunks * P, np.int64)
            idx[:ne] = _prow(s_g)
            idx_blocks.append(_wrap16(idx))
        per_core.append(
            dict(
                a8=a.astype(NP_FP8),
                oh8=oh.astype(NP_FP8),
                ohT8=ohT.astype(NP_FP8),
                src16=np.concatenate(idx_blocks, axis=1),
            )
        )

    # --- degree tensors (raw counts; device computes clip+rsqrt) -------------
    deg = []
    for c in range(NCORES):
        own_out = deg_out[c * NPC : (c + 1) * NPC]
        own_in = deg_in[c * NPC : (c + 1) * NPC]
        dout_blk = np.zeros((P, NB_BLK), np.float32)
        for b in range(NB_BLK):
            g0 = NPC * (b // NTILES) + P * (b % NTILES)
            w = min(P, NPC * (b // NTILES) + NPC - g0)
            dout_blk[:w, b] = deg_out[g0 : g0 + w]
        deg.append(
            dict(
                doutblk=dout_blk,
                dinbc=np.tile(own_in[None, :], (P, 1)),
                doutownbc=np.tile(own_out[None, :], (P, 1)),
            )
        )
    return per_core, deg, nchunks


# ----------------------------------------------------------------------------
# device program
# ----------------------------------------------------------------------------

def _build(nchunks):
    IDXW = nchunks * 8           # int16 idx cols per tile
    NBATCH = -(-nchunks // GB)   # GAT chunk batches per tile
    AC = 10                      # adjacency blocks per DMA chunk
    GSPLIT = [(0, 512), (512, 1024), (1024, NPC)]  # dst column groups

    nc = bacc.Bacc(
        "TRN2", target_bir_lowering=False, debug=False, num_devices=NCORES,
        num_swdge_queues=NQ,
    )

    _q = [0]

    def next_q():
        _q[0] = (_q[0] + 1) % NQ
        return _q[0]

    # --- I/O ----------------------------------------------------------------
    xtw_in = nc.dram_tensor("xtw", [P, 2 * NFULL], BF16, kind="ExternalInput")
    a8_in = nc.dram_tensor("a8", [P, NB_BLK * NPC], FP8, kind="ExternalInput")
    oh8_in = nc.dram_tensor(
        "oh8", [P, NTILES * nchunks * P], FP8, kind="ExternalInput"
    )
    oht8_in = nc.dram_tensor(
        "oht8", [P, NTILES * nchunks * P], FP8, kind="ExternalInput"
    )
    src16_in = nc.dram_tensor("src16", [P, NTILES * IDXW], I16, kind="ExternalInput")
    w1w_in = nc.dram_tensor("w1w", [P, 2 * HID], BF16, kind="ExternalInput")
    w2_in = nc.dram_tensor("w2", [P, HID], BF16, kind="ExternalInput")
    wghT_in = nc.dram_tensor("wghT", [P, HEADS * HID], F32, kind="ExternalInput")
    alT_in = nc.dram_tensor("alT", [P, HEADS], F32, kind="ExternalInput")
    arT_in = nc.dram_tensor("arT", [P, HEADS], F32, kind="ExternalInput")
    w3c_in = nc.dram_tensor("w3c", [P, 1], F32, kind="ExternalInput")
    w3bc_in = nc.dram_tensor("w3bc", [P, HID], F32, kind="ExternalInput")
    bgbc_in = nc.dram_tensor("bgbc", [P, HEADS * HID], F32, kind="ExternalInput")
    b1c_in = nc.dram_tensor("b1c", [P, 1], F32, kind="ExternalInput")
    b2c_in = nc.dram_tensor("b2c", [P, 1], F32, kind="ExternalInput")
    b3c_in = nc.dram_tensor("b3c", [P, 1], F32, kind="ExternalInput")
    doutblk_in = nc.dram_tensor("doutblk", [P, NB_BLK], F32, kind="ExternalInput")
    dinbc_in = nc.dram_tensor("dinbc", [P, NPC], F32, kind="ExternalInput")
    doutownbc_in = nc.dram_tensor("doutownbc", [P, NPC], F32, kind="ExternalInput")
    risk_out = nc.dram_tensor("risk", [NPC, 1], F32, kind="ExternalOutput")

    rg = [list(range(NCORES))]

    with tile.TileContext(nc) as tc:
        with (
            tc.tile_pool(name="const", bufs=1) as cp,
            tc.tile_pool(name="tab", bufs=1) as tabp,
            tc.tile_pool(name="a8", bufs=2) as a8p,
            tc.tile_pool(name="xs", bufs=2) as xp,
            tc.tile_pool(name="oh", bufs=2) as ohp,
            tc.tile_pool(name="oht", bufs=2) as ohtp,
            tc.tile_pool(name="gel", bufs=4) as gelp,
            tc.tile_pool(name="work", bufs=3) as wp,
            tc.tile_pool(name="acc", bufs=1, space="PSUM") as pacc,
            tc.tile_pool(name="pmm", bufs=2, space="PSUM") as pw,
            tc.tile_pool(name="pga", bufs=1, space="PSUM") as pga,
            tc.tile_pool(name="psm", bufs=2, space="PSUM") as psm,
            tc.tile_pool(name="dram", bufs=1, space="DRAM") as dram,
        ):
            # --- DRAM interchange buffers ---------------------------------
            ag2_in = dram.tile([NPAD, HID], BF16)
            tab2_d = dram.tile([NFULL, HID], BF16)
            ag3_in = dram.tile([P, NPAD], BF16)
            tab3t_d = dram.tile([NCORES * P, NPAD], BF16)
            ely_d = dram.tile([NFULL, HID], BF16)  # payload in cols 0:8
            ag4_in = dram.tile([NPAD, 1], BF16)
            tabs_d = dram.tile([NFULL, 1], BF16)

            # --- resident constants ---------------------------------------
            def cload(name, dram_t, shape, dt):
                t = cp.tile(shape, dt, tag=name)
                nc.sync.dma_start(out=t[:], in_=dram_t[:])
                return t

            w1w = cp.tile([P, 2, HID], BF16, tag="w1w")
            nc.sync.dma_start(
                out=w1w[:], in_=w1w_in[:].rearrange("p (k f) -> p k f", k=2)
            )
            w2 = cload("w2", w2_in, [P, HID], BF16)
            wghT = cload("wghT", wghT_in, [P, HEADS * HID], F32)
            alT = cload("alT", alT_in, [P, HEADS], F32)
            arT = cload("arT", arT_in, [P, HEADS], F32)
            w3c = cload("w3c", w3c_in, [P, 1], F32)
            w3bc = cload("w3bc", w3bc_in, [P, HID], F32)
            bgbc = cload("bgbc", bgbc_in, [P, HEADS * HID], F32)
            b1c = cload("b1c", b1c_in, [P, 1], F32)
            b2c = cload("b2c", b2c_in, [P, 1], F32)
            b3c = cload("b3c", b3c_in, [P, 1], F32)
            src16 = cload("src16", src16_in, [P, NTILES * IDXW], I16)

            ident = cp.tile([P, P], F32)
            make_identity(nc, ident[:])

            zrow = cp.tile([P, HID], BF16, tag="zrow")
            nc.vector.memset(zrow[:], 0.0)

            def rsqrt_inplace(t):
                nc.vector.tensor_scalar(
                    out=t[:], in0=t[:], scalar1=1.0, scalar2=None,
                    op0=mybir.AluOpType.max,
                )
                nc.vector.reciprocal(out=t[:], in_=t[:])
                nc.scalar.activation(
                    out=t[:], in_=t[:], func=mybir.ActivationFunctionType.Sqrt
                )
                return t

            dso = rsqrt_inplace(cload("doutblk", doutblk_in, [P, NB_BLK], F32))
            ddbc = rsqrt_inplace(cload("dinbc", dinbc_in, [P, NPC], F32))
            dsbc = rsqrt_inplace(cload("doutownbc", doutownbc_in, [P, NPC], F32))

            # dsrc for own nodes as per-tile columns: dscol[p, t] =
            # dsrc[t*128+p], read off the broadcast dsbc rows via a diagonal
            # mask + free-dim reduce.
            dscol = cp.tile([P, NTILES], F32, tag="dscol")
            for t in range(NTILES):
                w = TILE_W[t]
                tmp = wp.tile([P, P], F32, tag="diag")
                nc.vector.tensor_tensor(
                    out=tmp[:w, :w], in0=dsbc[:w, t * P : t * P + w],
                    in1=ident[:w, :w], op=mybir.AluOpType.mult,
                )
                nc.vector.reduce_sum(
                    out=dscol[:w, t : t + 1], in_=tmp[:w, :w],
                    axis=mybir.AxisListType.X,
                )

            # bgW3 = (mean_h bg_h) @ W3 as a [128, 1] broadcast column
            bgm = cp.tile([P, HID], F32, tag="bgm")
            nc.vector.tensor_tensor(
                out=bgm[:], in0=bgbc[:, 0:HID], in1=bgbc[:, HID : 2 * HID],
                op=mybir.AluOpType.add,
            )
            nc.vector.tensor_tensor(
                out=bgm[:], in0=bgm[:], in1=bgbc[:, 2 * HID : 3 * HID],
                op=mybir.AluOpType.add,
            )
            nc.vector.tensor_tensor(
                out=bgm[:], in0=bgm[:], in1=bgbc[:, 3 * HID : 4 * HID],
                op=mybir.AluOpType.add,
            )
            nc.vector.tensor_scalar(
                out=bgm[:], in0=bgm[:], scalar1=0.25, scalar2=None,
                op0=mybir.AluOpType.mult,
            )
            bgw3 = cp.tile([P, 1], F32, tag="bgw3")
            nc.vector.tensor_tensor(
                out=bgm[:], in0=bgm[:], in1=w3bc[:], op=mybir.AluOpType.mult
            )
            nc.vector.reduce_sum(out=bgw3[:], in_=bgm[:], axis=mybir.AxisListType.X)

            # ALY [128, 8] = [AL | AY], AR [128, 4]: per head h,
            # col = Wg_h^T-matmul with [alT_h | arT_h | W3]
            alyr_t = psm.tile([P, 512], F32, tag="small", space="PSUM")
            alyr_ps = alyr_t[:, 0:3]
            aly = cp.tile([P, 2 * HEADS], BF16, tag="aly")
            ar4 = cp.tile([P, HEADS], BF16, tag="ar4")
            for h in range(HEADS):
                rhs3 = wp.tile([P, 3], F32, tag="rhs3")
                nc.vector.tensor_copy(out=rhs3[:, 0:1], in_=alT[:, h : h + 1])
                nc.vector.tensor_copy(out=rhs3[:, 1:2], in_=arT[:, h : h + 1])
                nc.vector.tensor_copy(out=rhs3[:, 2:3], in_=w3c[:, 0:1])
                nc.tensor.matmul(
                    out=alyr_ps, lhsT=wghT[:, h * HID : (h + 1) * HID],
                    rhs=rhs3[:], start=True, stop=True,
                )
                nc.vector.tensor_copy(out=aly[:, h : h + 1], in_=alyr_t[:, 0:1])
                nc.vector.tensor_copy(out=ar4[:, h : h + 1], in_=alyr_t[:, 1:2])
                nc.vector.tensor_copy(
                    out=aly[:, HEADS + h : HEADS + h + 1], in_=alyr_t[:, 2:3]
                )

            # zero the pad rows of the DRAM AG buffers once
            nc.sync.dma_start(out=ag2_in[NPC:NPAD, :], in_=zrow[: NPAD - NPC, :])
            nc.sync.dma_start(
                out=ag4_in[NPC:NPAD, :], in_=zrow[: NPAD - NPC, 0:1]
            )

            # =============== GC1: replicated node transform ================
            # h1_full[n, f] = dsrc[n] * (x[n] @ W1), all 10240 padded rows.
            h1 = tabp.tile([P, NB_BLK, HID], BF16, tag="tab")
            XC = 10  # blocks per xT stream chunk
            for cc0 in range(0, NB_BLK, XC):
                xs = xp.tile([P, XC, 2, P], BF16, tag="xs")
                nc.sync.dma_start(
                    out=xs[:],
                    in_=xtw_in[
                        :, cc0 * 2 * P : (cc0 + XC) * 2 * P
                    ].rearrange("p (b k q) -> p b k q", k=2, q=P),
                )
                for bi in range(XC):
                    b = cc0 + bi
                    ps = pw.tile([P, 512], F32, tag="mmw", space="PSUM")
                    nc.tensor.matmul(
                        out=ps[:, 0:HID], lhsT=xs[:, bi, 0, :],
                        rhs=w1w[:, 0, :], start=True, stop=False,
                    )
                    nc.tensor.matmul(
                        out=ps[:, 0:HID], lhsT=xs[:, bi, 1, :],
                        rhs=w1w[:, 1, :], start=False, stop=True,
                    )
                    nc.vector.tensor_scalar(
                        out=h1[:, b, :], in0=ps[:, 0:HID],
                        scalar1=dso[:, b : b + 1],
                        scalar2=None, op0=mybir.AluOpType.mult,
                    )

            # =============== shared dense-aggregation pass =================
            def dense_agg(tab_sb):
                """aggT[f, d] accumulated over all 80 blocks; returns the
                3 PSUM group tiles (live until epilogue reads them)."""
                accs = [
                    pacc.tile(
                        [P, 512], F32, tag=f"acc{g}", space="PSUM",
                        name=f"acc{g}",
                    )
                    for g in range(3)
                ]
                for ac0 in range(0, NB_BLK, AC):
                    a8s = a8p.tile([P, AC, NPC], FP8, tag="a8s")
                    nc.sync.dma_start(
                        out=a8s[:],
                        in_=a8_in[:, ac0 * NPC : (ac0 + AC) * NPC].rearrange(
                            "p (b d) -> p b d", d=NPC
                        ),
                    )
                    for bi in range(AC):
                        b = ac0 + bi
                        for g, (g0, g1) in enumerate(GSPLIT):
                            nc.tensor.matmul(
                                out=accs[g][:, : g1 - g0],
                                lhsT=tab_sb[:, b, :],
                                rhs=a8s[:, bi, g0:g1],
                                start=(b == 0), stop=(b == NB_BLK - 1),
                                skip_group_check=True,
                            )
                return accs

            # =============== GC1 agg + GC2 node ===========================
            accs = dense_agg(h1)
            x2s = wp.tile([P, NPC], BF16, tag="x2s")
            for g, (g0, g1) in enumerate(GSPLIT):
                gw = g1 - g0
                t1 = wp.tile([P, 512], F32, tag="epi1")
                nc.vector.tensor_tensor(
                    out=t1[:, :gw], in0=accs[g][:, :gw], in1=ddbc[:, g0:g1],
                    op=mybir.AluOpType.mult,
                )
                nc.scalar.activation(
                    out=t1[:, :gw], in_=t1[:, :gw],
                    func=mybir.ActivationFunctionType.Relu,
                    bias=b1c[:, 0:1], scale=1.0,
                )
                nc.vector.tensor_tensor(
                    out=x2s[:, g0:g1], in0=t1[:, :gw], in1=dsbc[:, g0:g1],
                    op=mybir.AluOpType.mult,
                )
            # h2T[f2, d] = W2^T @ x2s
            h2t = wp.tile([P, NPC], F32, tag="h2t")
            for g, (g0, g1) in enumerate(GSPLIT):
                ps = pw.tile([P, 512], F32, tag="mmw", space="PSUM")
                nc.tensor.matmul(
                    out=ps[:, : g1 - g0], lhsT=w2[:], rhs=x2s[:, g0:g1],
                    start=True, stop=True,
                )
                nc.vector.tensor_copy(out=h2t[:, g0:g1], in_=ps[:, : g1 - g0])
            # transpose h2T tiles -> row-major h2 shard -> AG2 buffer
            for t in range(NTILES):
                w = TILE_W[t]
                pt = pw.tile([P, 512], F32, tag="mmw", space="PSUM")
                nc.tensor.transpose(
                    out=pt[:w, 0:P], in_=h2t[:, t * P : t * P + w],
                    identity=ident[:],
                )
                h2r = wp.tile([P, HID], BF16, tag="h2r")
                nc.vector.tensor_copy(out=h2r[:w, :], in_=pt[:w, 0:P])
                nc.sync.dma_start(
                    out=ag2_in[t * P : t * P + w, :], in_=h2r[:w, :]
                )

            nc.gpsimd.collective_compute(
                "AllGather", mybir.AluOpType.bypass, replica_groups=rg,
                ins=[ag2_in[:].opt()], outs=[tab2_d[:].opt()],
            )

            # =============== GC2 agg + GAT node prep ======================
            tab2 = tabp.tile([P, NB_BLK, HID], BF16, tag="tab")
            nc.sync.dma_start(
                out=tab2[:], in_=tab2_d[:].rearrange("(b p) f -> p b f", p=P)
            )
            accs = dense_agg(tab2)
            # x3T [f, d] bf16 (padded cols zeroed for the AG)
            x3t = cp.tile([P, NPAD], BF16, tag="x3t")
            nc.vector.memset(x3t[:, NPC:NPAD], 0.0)
            for g, (g0, g1) in enumerate(GSPLIT):
                gw = g1 - g0
                t1 = wp.tile([P, 512], F32, tag="epi1")
                nc.vector.tensor_tensor(
                    out=t1[:, :gw], in0=accs[g][:, :gw], in1=ddbc[:, g0:g1],
                    op=mybir.AluOpType.mult,
                )
                nc.scalar.activation(
                    out=x3t[:, g0:g1], in_=t1[:, :gw],
                    func=mybir.ActivationFunctionType.Relu,
                    bias=b2c[:, 0:1], scale=1.0,
                )
            nc.sync.dma_start(out=ag3_in[:], in_=x3t[:])
            # er per dst tile: [d, 4] = x3T_tile^T @ AR
            er_sb = cp.tile([P, NTILES * HEADS], BF16, tag="er_sb")
            nc.vector.memset(er_sb[:], 0.0)
            for t in range(NTILES):
                w = TILE_W[t]
                ps = psm.tile([P, 512], F32, tag="small", space="PSUM")
                nc.tensor.matmul(
                    out=ps[:w, 0:HEADS], lhsT=x3t[:, t * P : t * P + w],
                    rhs=ar4[:], start=True, stop=True,
                )
                nc.vector.tensor_copy(
                    out=er_sb[:w, t * HEADS : (t + 1) * HEADS],
                    in_=ps[:w, 0:HEADS],
                )

            nc.gpsimd.collective_compute(
                "AllGather", mybir.AluOpType.bypass, replica_groups=rg,
                ins=[ag3_in[:].opt()], outs=[tab3t_d[:].opt()],
            )

            # =============== ely table: [el|y] per node ===================
            tab3t = tabp.tile([P, NCORES, NTILES, P], BF16, tag="tab")
            nc.sync.dma_start(
                out=tab3t[:],
                in_=tab3t_d[:].rearrange(
                    "(c p) (j q) -> p c j q", p=P, q=P
                ),
            )
            ely = cp.tile([P, NB_BLK, 8], BF16, tag="ely")
            for g in range(2):
                ps = psm.tile([P, 512], F32, tag="small", space="PSUM")
                for bb in range(40):
                    b = g * 40 + bb
                    nc.tensor.matmul(
                        out=ps[:, bb * 8 : bb * 8 + 8],
                        lhsT=tab3t[:, b // NTILES, b % NTILES, :],
                        rhs=aly[:], start=True, stop=True,
                        skip_group_check=True,
                    )
                nc.vector.tensor_copy(
                    out=ely[:, g * 40 : (g + 1) * 40, :],
                    in_=ps[:, 0:320].rearrange("p (b f) -> p b f", f=8),
                )
            nc.sync.dma_start(
                out=ely_d[:, 0:8].rearrange("(b p) f -> p b f", p=P),
                in_=ely[:],
            )

            # =============== GAT edge phase ===============================
            for t in range(NTILES):
                w = TILE_W[t]
                gel = gelp.tile([P, nchunks, HID], BF16, tag="gel")
                qstep = -(-nchunks // 4)
                for c0 in range(0, nchunks, qstep):
                    c1 = min(c0 + qstep, nchunks)
                    nc.gpsimd.dma_gather(
                        gel[:, c0:c1, :], ely_d[:],
                        src16[:, t * IDXW + c0 * 8 : t * IDXW + c1 * 8],
                        (c1 - c0) * P, (c1 - c0) * P, HID, elem_step=HID,
                        single_packet=False, queue_num=next_q(),
                    )
                oh8 = ohp.tile([P, nchunks, P], FP8, tag="oh8")
                nc.scalar.dma_start(
                    out=oh8[:],
                    in_=oh8_in[
                        :, t * nchunks * P : (t + 1) * nchunks * P
                    ].rearrange("p (c d) -> p c d", d=P),
                )
                oht8 = ohtp.tile([P, nchunks, P], FP8, tag="oht8")
                nc.scalar.dma_start(
                    out=oht8[:],
                    in_=oht8_in[
                        :, t * nchunks * P : (t + 1) * nchunks * P
                    ].rearrange("p (c e) -> p c e", e=P),
                )
                acc_t = pga.tile([P, 512], F32, tag="gacc", space="PSUM")
                acc = acc_t[:, 0:8]
                ert = er_sb[:, t * HEADS : (t + 1) * HEADS]
                for b0 in range(0, nchunks, GB):
                    b1 = min(b0 + GB, nchunks)
                    nb = b1 - b0
                    erp_t = psm.tile([P, 512], F32, tag="small", space="PSUM")
                    erp = erp_t[:, 0 : GB * HEADS].rearrange(
                        "p (c h) -> p c h", h=HEADS
                    )
                    for cc in range(b0, b1):
                        nc.tensor.matmul(
                            out=erp[:, cc - b0, :], lhsT=oht8[:, cc, :],
                            rhs=ert, start=True, stop=True,
                            skip_group_check=True,
                        )
                    # e = lrelu(el + er); ex = exp(e)
                    e_all = wp.tile([P, GB, HEADS], F32, tag="e_all")
                    nc.vector.tensor_tensor(
                        out=e_all[:, :nb, :], in0=gel[:, b0:b1, 0:HEADS],
                        in1=erp[:, :nb, :], op=mybir.AluOpType.add,
                    )
                    nc.vector.scalar_tensor_tensor(
                        out=e_all[:, :nb, :], in0=e_all[:, :nb, :], scalar=0.2,
                        in1=e_all[:, :nb, :], op0=mybir.AluOpType.mult,
                        op1=mybir.AluOpType.max,
                    )
                    # rhs = [y*ex | ex] bf16; exp lands in rp directly
                    rp = wp.tile([P, GB, 8], BF16, tag="rp")
                    nc.scalar.activation(
                        out=rp[:, :nb, HEADS:8], in_=e_all[:, :nb, :],
                        func=mybir.ActivationFunctionType.Exp,
                    )
                    nc.vector.tensor_tensor(
                        out=rp[:, :nb, 0:HEADS], in0=gel[:, b0:b1, HEADS:8],
                        in1=rp[:, :nb, HEADS:8], op=mybir.AluOpType.mult,
                    )
                    for cc in range(b0, b1):
                        nc.tensor.matmul(
                            out=acc, lhsT=oh8[:, cc, :],
                            rhs=rp[:, cc - b0, :],
                            start=(cc == 0), stop=(cc == nchunks - 1),
                            skip_group_check=True,
                        )
                # epilogue: s = dsrc * (mean_h(yagg/den) + bgW3)
                den = wp.tile([P, HEADS], F32, tag="den")
                nc.vector.tensor_scalar(
                    out=den[:], in0=acc_t[:, HEADS:8], scalar1=1e-30,
                    scalar2=None, op0=mybir.AluOpType.max,
                )
                nc.vector.reciprocal(out=den[:], in_=den[:])
                wy = wp.tile([P, HEADS], F32, tag="wy")
                nc.vector.tensor_tensor(
                    out=wy[:], in0=acc_t[:, 0:HEADS], in1=den[:],
                    op=mybir.AluOpType.mult,
                )
                sv = wp.tile([P, 1], F32, tag="sv")
                nc.vector.reduce_sum(out=sv[:], in_=wy[:], axis=mybir.AxisListType.X)
                nc.vector.scalar_tensor_tensor(
                    out=sv[:], in0=sv[:], scalar=0.25, in1=bgw3[:],
                    op0=mybir.AluOpType.mult, op1=mybir.AluOpType.add,
                )
                svb = wp.tile([P, 1], BF16, tag="svb")
                nc.vector.tensor_scalar(
                    out=svb[:], in0=sv[:], scalar1=dscol[:, t : t + 1],
                    scalar2=None, op0=mybir.AluOpType.mult,
                )
                nc.sync.dma_start(
                    out=ag4_in[t * P : t * P + w, :], in_=svb[:w, :]
                )

            # prefetch the first two GC3 adjacency chunks during AG4
            a8pre = []
            for pc in range(2):
                a8s = a8p.tile([P, AC, NPC], FP8, tag="a8s", name=f"a8pre{pc}")
                nc.scalar.dma_start(
                    out=a8s[:],
                    in_=a8_in[:, pc * AC * NPC : (pc + 1) * AC * NPC].rearrange(
                        "p (b d) -> p b d", d=NPC
                    ),
                )
                a8pre.append(a8s)

            nc.gpsimd.collective_compute(
                "AllGather", mybir.AluOpType.bypass, replica_groups=rg,
                ins=[ag4_in[:].opt()], outs=[tabs_d[:].opt()],
            )

            # =============== GC3: dense matvec + sigmoid ==================
            s_sb = cp.tile([P, NB_BLK], BF16, tag="s_sb")
            nc.sync.dma_start(
                out=s_sb[:], in_=tabs_d[:].rearrange("(b p) one -> p (b one)", p=P)
            )
            acc3 = [
                pacc.tile(
                    [P, 512], F32, tag=f"acc{g}", space="PSUM", name=f"acc3{g}"
                )
                for g in range(3)
            ]
            for ac0 in range(0, NB_BLK, AC):
                ci = ac0 // AC
                if ci < 2:
                    a8s = a8pre[ci]
                else:
                    a8s = a8p.tile([P, AC, NPC], FP8, tag="a8s")
                    nc.sync.dma_start(
                        out=a8s[:],
                        in_=a8_in[:, ac0 * NPC : (ac0 + AC) * NPC].rearrange(
                            "p (b d) -> p b d", d=NPC
                        ),
                    )
                for bi in range(AC):
                    b = ac0 + bi
                    for g, (g0, g1) in enumerate(GSPLIT):
                        nc.tensor.matmul(
                            out=acc3[g][0:1, : g1 - g0],
                            lhsT=s_sb[:, b : b + 1],
                            rhs=a8s[:, bi, g0:g1],
                            start=(b == 0), stop=(b == NB_BLK - 1),
                            skip_group_check=True,
                        )
            risk_sb = wp.tile([P, NPC], F32, tag="risk")
            for g, (g0, g1) in enumerate(GSPLIT):
                gw = g1 - g0
                nc.vector.tensor_tensor(
                    out=risk_sb[0:1, g0:g1], in0=acc3[g][0:1, :gw],
                    in1=ddbc[0:1, g0:g1], op=mybir.AluOpType.mult,
                )
            nc.scalar.activation(
                out=risk_sb[0:1, :], in_=risk_sb[0:1, :],
                func=mybir.ActivationFunctionType.Sigmoid,
                bias=b3c[0:1, 0:1], scale=1.0,
            )
            nc.sync.dma_start(
                out=risk_out[:].rearrange("d one -> one d"),
                in_=risk_sb[0:1, :],
            )

    nc.compile()
    return nc


# ----------------------------------------------------------------------------
# host driver
# ----------------------------------------------------------------------------

def _get_program(nchunks):
    if nchunks not in _compiled_cache:
        _compiled_cache[nchunks] = _build(nchunks)
    return _compiled_cache[nchunks]


def _install_ntff_hook():
    """Profiling support: register the NTFF hook bass_utils expects when this
    image's antenv package lacks axon_hooks. Best-effort, trace-path only."""
    import types

    try:
        import antenv.axon_hooks  # noqa: F401

        return
    except ImportError:
        pass
    try:
        import antenv
        from trn_agent_boot.trn_boot import _ntff_profile_via_ctypes

        hook = _ntff_profile_via_ctypes("/opt/axon/libaxon_pjrt.so")
        mod = types.ModuleType("antenv.axon_hooks")
        mod.get_axon_ntff_profile_hook = lambda: hook
        mod.set_axon_ntff_profile_hook = lambda h: None
        sys.modules["antenv.axon_hooks"] = mod
        antenv.axon_hooks = mod
    except Exception:
        pass


def kernel(
    features, src, dst, W1, b1, W2, b2, W3, b3, Wg, attn_l, attn_r, bg,
    _trace=False,
):
    features = np.asarray(features, np.float32)
    per_core, deg, nchunks = _preprocess(src, dst)
    nc = _get_program(nchunks)

    # full features, wrapped + padded per 128-row block:
    # xtw[p, b, k, q] = x[row b*128+q, k*128+p]
    xpad = np.zeros((NFULL, IN_F), np.float32)
    for c in range(NCORES):
        xpad[c * NPAD : c * NPAD + NPC] = features[c * NPC : (c + 1) * NPC]
    xtw = (
        xpad.reshape(NB_BLK, P, 2, P)        # [b, q, k, p]
        .transpose(3, 0, 2, 1)               # [p, b, k, q]
        .reshape(P, 2 * NFULL)
        .astype(NP_BF16)
    )

    W1 = np.asarray(W1, np.float32)
    w1w = np.concatenate([W1[:P, :], W1[P:, :]], axis=1).astype(NP_BF16)
    Wg = np.asarray(Wg, np.float32)
    wghT = np.zeros((P, HEADS * HID), np.float32)
    for h in range(HEADS):
        wghT[:, h * HID : (h + 1) * HID] = Wg[:, h * HID : (h + 1) * HID].T

    common = dict(
        xtw=xtw,
        w1w=w1w,
        w2=np.asarray(W2, np.float32).astype(NP_BF16),
        wghT=wghT,
        alT=np.asarray(attn_l, np.float32).T.copy(),
        arT=np.asarray(attn_r, np.float32).T.copy(),
        w3c=np.asarray(W3, np.float32).reshape(P, 1),
        w3bc=np.tile(np.asarray(W3, np.float32).reshape(1, -1), (P, 1)),
        bgbc=np.tile(np.asarray(bg, np.float32).reshape(1, -1), (P, 1)),
        b1c=np.asarray(b1, np.float32).reshape(P, 1),
        b2c=np.asarray(b2, np.float32).reshape(P, 1),
        b3c=np.full((P, 1), np.float32(np.asarray(b3).reshape(-1)[0])),
    )
    in_maps = []
    for c in range(NCORES):
        m = dict(common)
        m["a8"] = per_core[c]["a8"]
        m["oh8"] = per_core[c]["oh8"]
        m["oht8"] = per_core[c]["ohT8"]
        m["src16"] = per_core[c]["src16"]
        m["doutblk"] = deg[c]["doutblk"]
        m["dinbc"] = deg[c]["dinbc"]
        m["doutownbc"] = deg[c]["doutownbc"]
        in_maps.append(m)

    if _trace:
        _install_ntff_hook()
    res = bass_utils.run_bass_kernel_spmd(
        nc, in_maps, core_ids=list(range(NCORES)), trace=_trace
    )
    out = np.concatenate([res.results[c]["risk"] for c in range(NCORES)], axis=0)
    if _trace:
        kernel.last_exec_time_ns = res.exec_time_ns
        kernel.last_results = res
    return out.astype(np.float32)


# revision 15
# speedup vs baseline: 1.1692x; 1.0893x over previous
"""Trainium2 Bass kernel for a 4-layer dependency GNN (3x GraphConv + GAT).

Full inputs in, full output out. Internally nodes are sharded across 8
NeuronCores by dst ownership (1250 nodes/core, padded to 1280 = 10 blocks
of 128 per shard).

Design (v2 — dense-adjacency):
  - The per-core adjacency A_c [10000 src x 1250 dst] (entry = edge
    multiplicity) is built on the host from the integer edge list and
    uploaded as fp8 (0/1/2 are exact).  GraphConv aggregations run as dense
    matmuls: aggT[f, d] = sum_b h_blk[b]^T @ A_blk[b]  (lhsT = 128-row
    table block, rhs = fp8 A block streamed from DRAM).  No per-edge
    gather and no one-hot builds for any GraphConv layer.
  - GC1 is fully replicated: every core receives the full feature matrix
    (bf16) and computes the full h1 table locally -> no AllGather before
    the first aggregation.
  - GAT: the GAT output is only consumed through mean_h(out)@W3, which is
    linear, so W3 is folded through the attention: per edge only
    y_h = x3 @ (Wg_h @ W3) (4 scalars) and ex_h (4 scalars) are
    aggregated.  Per-node [el|y] rows (16 B) are computed densely and
    per-edge rows fetched with one dma_gather per dst tile; the dst-side
    er term is expanded with transposed one-hot matmuls.  One-hot matrices
    (0/1) are uploaded from the host in fp8 and streamed.
  - Per-layer cross-core tables move through 4 small AllGathers
    (320 KB x 3 + 2.5 KB).

Host-side work is limited to index manipulation (edge bucketing, one-hot /
adjacency construction from integer indices, wrapping/padding, integer
degree counts) and dtype casts; all floating-point model math runs on
device (degree^-1/2 included).
"""

import sys

import numpy as np

sys.path.insert(0, "/opt/trn_rl_repo")

import ml_dtypes  # noqa: E402

import concourse.bacc as bacc  # noqa: E402
import concourse.mybir as mybir  # noqa: E402
import concourse.tile as tile  # noqa: E402
from concourse import bass_utils  # noqa: E402
from concourse.masks import make_identity  # noqa: E402

N = 10000
E = 320000
IN_F = 256
HID = 128
HEADS = 4
NCORES = 8
NPC = N // NCORES           # nodes per core (1250)
P = 128
NTILES = (NPC + P - 1) // P  # dst tiles per core (10)
TILE_W = [min(P, NPC - t * P) for t in range(NTILES)]
NPAD = NTILES * P            # padded shard rows (1280)
NB_BLK = NCORES * NTILES     # global 128-row src blocks (80)
NFULL = NCORES * NPAD        # padded table rows (10240)
GB = 7                       # chunk batch in the GAT edge phase
NQ = 4                       # SWDGE queues

F32 = mybir.dt.float32
BF16 = mybir.dt.bfloat16
FP8 = mybir.dt.float8e4
I16 = mybir.dt.int16

NP_BF16 = ml_dtypes.bfloat16
NP_FP8 = ml_dtypes.float8_e4m3

_compiled_cache = {}


# ----------------------------------------------------------------------------
# host-side sharding / index preprocessing (integer work + dtype casts only)
# ----------------------------------------------------------------------------

def _wrap16(idx_block):
    """dma_gather index layout: [16, n/16] with [p, s] = idx[s*16+p],
    replicated across the 8 gpsimd cores (8 groups of 16 partitions)."""
    n = idx_block.shape[0]
    assert n % 16 == 0
    base = idx_block.reshape(n // 16, 16).T.astype(np.int16)
    return np.tile(base, (8, 1))


def _prow(n):
    """Row of global node n in the 10240-row padded table."""
    return NPAD * (n // NPC) + (n % NPC)


def _preprocess(src, dst):
    src = np.asarray(src).astype(np.int64).ravel()
    dst = np.asarray(dst).astype(np.int64).ravel()

    deg_out = np.bincount(src, minlength=N).astype(np.float32)
    deg_in = np.bincount(dst, minlength=N).astype(np.float32)

    # --- bucket edges by (dst core, dst tile) --------------------------------
    groups = {}
    counts = np.zeros((NCORES, NTILES), np.int64)
    for c in range(NCORES):
        sel = (dst // NPC) == c
        s_c = src[sel]
        d_c = dst[sel] - c * NPC
        order = np.argsort(d_c, kind="stable")
        s_c, d_c = s_c[order], d_c[order]
        t_c = d_c // P
        for t in range(NTILES):
            m = t_c == t
            groups[(c, t)] = (s_c[m], d_c[m] - t * P)
            counts[c, t] = int(m.sum())
    nchunks = int(-(-counts.max() // P))

    per_core = []
    for c in range(NCORES):
        # adjacency: [128, NB_BLK * NPC] fp8; block b covers global src rows
        # [NPC*(b//NTILES) + P*(b%NTILES), +128) (rows beyond the 98-wide
        # tail blocks stay zero).
        a = np.zeros((P, NB_BLK * NPC), np.float32)
        # one-hots for the GAT edge phase: [128, NTILES*nchunks*128]
        oh = np.zeros((P, NTILES * nchunks * P), np.float32)
        ohT = np.zeros((P, NTILES * nchunks * P), np.float32)
        idx_blocks = []
        for t in range(NTILES):
            s_g, dl_g = groups[(c, t)]
            ne = len(s_g)
            # adjacency entries for this tile's edges
            sl = s_g % NPC
            blk = NTILES * (s_g // NPC) + sl // P
            srow = sl % P
            np.add.at(a, (srow, blk * NPC + t * P + dl_g), 1.0)
            # per-edge slots: edge i -> chunk i//128, lane i%128
            ch = np.arange(ne) // P
            lane = np.arange(ne) % P
            base = (t * nchunks + ch) * P
            oh[lane, base + dl_g] = 1.0
            ohT[dl_g, base + lane] = 1.0
            # gather indices (padded rows use index 0 -> finite garbage,
            # masked by zero one-hot columns)
            idx = np.zeros(nchunks * P, np.int64)
            idx[:ne] = _prow(s_g)
            idx_blocks.append(_wrap16(idx))
        per_core.append(
            dict(
                a8=a.astype(NP_FP8),
                oh8=oh.astype(NP_FP8),
                ohT8=ohT.astype(NP_FP8),
                src16=np.concatenate(idx_blocks, axis=1),
            )
        )

    # --- degree tensors (raw counts; device computes clip+rsqrt) -------------
    deg = []
    for c in range(NCORES):
        own_out = deg_out[c * NPC : (c + 1) * NPC]
        own_in = deg_in[c * NPC : (c + 1) * NPC]
        dout_blk = np.zeros((P, NB_BLK), np.float32)
        for b in range(NB_BLK):
            g0 = NPC * (b // NTILES) + P * (b % NTILES)
            w = min(P, NPC * (b // NTILES) + NPC - g0)
            dout_blk[:w, b] = deg_out[g0 : g0 + w]
        deg.append(
            dict(
                doutblk=dout_blk,
                dinbc=np.tile(own_in[None, :], (P, 1)),
                doutownbc=np.tile(own_out[None, :], (P, 1)),
            )
        )
    return per_core, deg, nchunks


# ----------------------------------------------------------------------------
# device program
# ----------------------------------------------------------------------------

def _build(nchunks):
    IDXW = nchunks * 8           # int16 idx cols per tile
    NBATCH = -(-nchunks // GB)   # GAT chunk batches per tile
    AC = 10                      # adjacency blocks per DMA chunk
    GSPLIT = [(0, 512), (512, 1024), (1024, NPC)]  # dst column groups

    nc = bacc.Bacc(
        "TRN2", target_bir_lowering=False, debug=False, num_devices=NCORES,
        num_swdge_queues=NQ,
    )

    _q = [0]

    def next_q():
        _q[0] = (_q[0] + 1) % NQ
        return _q[0]

    # --- I/O ----------------------------------------------------------------
    xtw_in = nc.dram_tensor("xtw", [P, 2 * NFULL], BF16, kind="ExternalInput")
    a8_in = nc.dram_tensor("a8", [P, NB_BLK * NPC], FP8, kind="ExternalInput")
    oh8_in = nc.dram_tensor(
        "oh8", [P, NTILES * nchunks * P], FP8, kind="ExternalInput"
    )
    oht8_in = nc.dram_tensor(
        "oht8", [P, NTILES * nchunks * P], FP8, kind="ExternalInput"
    )
    src16_in = nc.dram_tensor("src16", [P, NTILES * IDXW], I16, kind="ExternalInput")
    w1w_in = nc.dram_tensor("w1w", [P, 2 * HID], BF16, kind="ExternalInput")
    w2_in = nc.dram_tensor("w2", [P, HID], BF16, kind="ExternalInput")
    wghT_in = nc.dram_tensor("wghT", [P, HEADS * HID], F32, kind="ExternalInput")
    alT_in = nc.dram_tensor("alT", [P, HEADS], F32, kind="ExternalInput")
    arT_in = nc.dram_tensor("arT", [P, HEADS], F32, kind="ExternalInput")
    w3c_in = nc.dram_tensor("w3c", [P, 1], F32, kind="ExternalInput")
    w3bc_in = nc.dram_tensor("w3bc", [P, HID], F32, kind="ExternalInput")
    bgbc_in = nc.dram_tensor("bgbc", [P, HEADS * HID], F32, kind="ExternalInput")
    b1c_in = nc.dram_tensor("b1c", [P, 1], F32, kind="ExternalInput")
    b2c_in = nc.dram_tensor("b2c", [P, 1], F32, kind="ExternalInput")
    b3c_in = nc.dram_tensor("b3c", [P, 1], F32, kind="ExternalInput")
    doutblk_in = nc.dram_tensor("doutblk", [P, NB_BLK], F32, kind="ExternalInput")
    dinbc_in = nc.dram_tensor("dinbc", [P, NPC], F32, kind="ExternalInput")
    doutownbc_in = nc.dram_tensor("doutownbc", [P, NPC], F32, kind="ExternalInput")
    risk_out = nc.dram_tensor("risk", [NPC, 1], F32, kind="ExternalOutput")

    rg = [list(range(NCORES))]

    with tile.TileContext(nc) as tc:
        with (
            tc.tile_pool(name="const", bufs=1) as cp,
            tc.tile_pool(name="tab", bufs=1) as tabp,
            tc.tile_pool(name="a8", bufs=2) as a8p,
            tc.tile_pool(name="xs", bufs=2) as xp,
            tc.tile_pool(name="oh", bufs=2) as ohp,
            tc.tile_pool(name="oht", bufs=2) as ohtp,
            tc.tile_pool(name="gel", bufs=4) as gelp,
            tc.tile_pool(name="work", bufs=3) as wp,
            tc.tile_pool(name="acc", bufs=1, space="PSUM") as pacc,
            tc.tile_pool(name="pmm", bufs=2, space="PSUM") as pw,
            tc.tile_pool(name="pga", bufs=1, space="PSUM") as pga,
            tc.tile_pool(name="psm", bufs=2, space="PSUM") as psm,
            tc.tile_pool(name="dram", bufs=1, space="DRAM") as dram,
        ):
            # --- DRAM interchange buffers ---------------------------------
            ag2_in = dram.tile([NPAD, HID], BF16)
            tab2_d = dram.tile([NFULL, HID], BF16, addr_space="Shared")
            ag3_in = dram.tile([P, NPAD], BF16)
            tab3t_d = dram.tile([NCORES * P, NPAD], BF16, addr_space="Shared")
            ely_d = dram.tile([NFULL, HID], BF16)  # payload in cols 0:8
            ag4_in = dram.tile([NPAD, 1], BF16)
            tabs_d = dram.tile([NFULL, 1], BF16, addr_space="Shared")

            # --- resident constants ---------------------------------------
            def cload(name, dram_t, shape, dt):
                t = cp.tile(shape, dt, tag=name)
                nc.sync.dma_start(out=t[:], in_=dram_t[:])
                return t

            w1w = cp.tile([P, 2, HID], BF16, tag="w1w")
            nc.sync.dma_start(
                out=w1w[:], in_=w1w_in[:].rearrange("p (k f) -> p k f", k=2)
            )
            w2 = cload("w2", w2_in, [P, HID], BF16)
            wghT = cload("wghT", wghT_in, [P, HEADS * HID], F32)
            alT = cload("alT", alT_in, [P, HEADS], F32)
            arT = cload("arT", arT_in, [P, HEADS], F32)
            w3c = cload("w3c", w3c_in, [P, 1], F32)
            w3bc = cload("w3bc", w3bc_in, [P, HID], F32)
            bgbc = cload("bgbc", bgbc_in, [P, HEADS * HID], F32)
            b1c = cload("b1c", b1c_in, [P, 1], F32)
            b2c = cload("b2c", b2c_in, [P, 1], F32)
            b3c = cload("b3c", b3c_in, [P, 1], F32)
            src16 = cload("src16", src16_in, [P, NTILES * IDXW], I16)

            ident = cp.tile([P, P], F32)
            make_identity(nc, ident[:])

            zrow = cp.tile([P, HID], BF16, tag="zrow")
            nc.vector.memset(zrow[:], 0.0)

            def rsqrt_inplace(t):
                nc.vector.tensor_scalar(
                    out=t[:], in0=t[:], scalar1=1.0, scalar2=None,
                    op0=mybir.AluOpType.max,
                )
                nc.vector.reciprocal(out=t[:], in_=t[:])
                nc.scalar.activation(
                    out=t[:], in_=t[:], func=mybir.ActivationFunctionType.Sqrt
                )
                return t

            dso = rsqrt_inplace(cload("doutblk", doutblk_in, [P, NB_BLK], F32))
            ddbc = rsqrt_inplace(cload("dinbc", dinbc_in, [P, NPC], F32))
            dsbc = rsqrt_inplace(cload("doutownbc", doutownbc_in, [P, NPC], F32))

            # dsrc for own nodes as per-tile columns: dscol[p, t] =
            # dsrc[t*128+p], read off the broadcast dsbc rows via a diagonal
            # mask + free-dim reduce.
            dscol = cp.tile([P, NTILES], F32, tag="dscol")
            for t in range(NTILES):
                w = TILE_W[t]
                tmp = wp.tile([P, P], F32, tag="diag")
                nc.vector.tensor_tensor(
                    out=tmp[:w, :w], in0=dsbc[:w, t * P : t * P + w],
                    in1=ident[:w, :w], op=mybir.AluOpType.mult,
                )
                nc.vector.reduce_sum(
                    out=dscol[:w, t : t + 1], in_=tmp[:w, :w],
                    axis=mybir.AxisListType.X,
                )

            # bgW3 = (mean_h bg_h) @ W3 as a [128, 1] broadcast column
            bgm = cp.tile([P, HID], F32, tag="bgm")
            nc.vector.tensor_tensor(
                out=bgm[:], in0=bgbc[:, 0:HID], in1=bgbc[:, HID : 2 * HID],
                op=mybir.AluOpType.add,
            )
            nc.vector.tensor_tensor(
                out=bgm[:], in0=bgm[:], in1=bgbc[:, 2 * HID : 3 * HID],
                op=mybir.AluOpType.add,
            )
            nc.vector.tensor_tensor(
                out=bgm[:], in0=bgm[:], in1=bgbc[:, 3 * HID : 4 * HID],
                op=mybir.AluOpType.add,
            )
            nc.vector.tensor_scalar(
                out=bgm[:], in0=bgm[:], scalar1=0.25, scalar2=None,
                op0=mybir.AluOpType.mult,
            )
            bgw3 = cp.tile([P, 1], F32, tag="bgw3")
            nc.vector.tensor_tensor(
                out=bgm[:], in0=bgm[:], in1=w3bc[:], op=mybir.AluOpType.mult
            )
            nc.vector.reduce_sum(out=bgw3[:], in_=bgm[:], axis=mybir.AxisListType.X)

            # ALY [128, 8] = [AL | AY], AR [128, 4]: per head h,
            # col = Wg_h^T-matmul with [alT_h | arT_h | W3]
            alyr_t = psm.tile([P, 512], F32, tag="small", space="PSUM")
            alyr_ps = alyr_t[:, 0:3]
            aly = cp.tile([P, 2 * HEADS], BF16, tag="aly")
            ar4 = cp.tile([P, HEADS], BF16, tag="ar4")
            for h in range(HEADS):
                rhs3 = wp.tile([P, 3], F32, tag="rhs3")
                nc.vector.tensor_copy(out=rhs3[:, 0:1], in_=alT[:, h : h + 1])
                nc.vector.tensor_copy(out=rhs3[:, 1:2], in_=arT[:, h : h + 1])
                nc.vector.tensor_copy(out=rhs3[:, 2:3], in_=w3c[:, 0:1])
                nc.tensor.matmul(
                    out=alyr_ps, lhsT=wghT[:, h * HID : (h + 1) * HID],
                    rhs=rhs3[:], start=True, stop=True,
                )
                nc.vector.tensor_copy(out=aly[:, h : h + 1], in_=alyr_t[:, 0:1])
                nc.vector.tensor_copy(out=ar4[:, h : h + 1], in_=alyr_t[:, 1:2])
                nc.vector.tensor_copy(
                    out=aly[:, HEADS + h : HEADS + h + 1], in_=alyr_t[:, 2:3]
                )

            # zero the pad rows of the DRAM AG buffers once
            nc.sync.dma_start(out=ag2_in[NPC:NPAD, :], in_=zrow[: NPAD - NPC, :])
            nc.sync.dma_start(
                out=ag4_in[NPC:NPAD, :], in_=zrow[: NPAD - NPC, 0:1]
            )

            # =============== GC1: replicated node transform ================
            # h1_full[n, f] = dsrc[n] * (x[n] @ W1), all 10240 padded rows.
            h1 = tabp.tile([P, NB_BLK, HID], BF16, tag="tab")
            XC = 10  # blocks per xT stream chunk
            for cc0 in range(0, NB_BLK, XC):
                xs = xp.tile([P, XC, 2, P], BF16, tag="xs")
                nc.sync.dma_start(
                    out=xs[:],
                    in_=xtw_in[
                        :, cc0 * 2 * P : (cc0 + XC) * 2 * P
                    ].rearrange("p (b k q) -> p b k q", k=2, q=P),
                )
                for bi in range(XC):
                    b = cc0 + bi
                    ps = pw.tile([P, 512], F32, tag="mmw", space="PSUM")
                    nc.tensor.matmul(
                        out=ps[:, 0:HID], lhsT=xs[:, bi, 0, :],
                        rhs=w1w[:, 0, :], start=True, stop=False,
                    )
                    nc.tensor.matmul(
                        out=ps[:, 0:HID], lhsT=xs[:, bi, 1, :],
                        rhs=w1w[:, 1, :], start=False, stop=True,
                    )
                    nc.vector.tensor_scalar(
                        out=h1[:, b, :], in0=ps[:, 0:HID],
                        scalar1=dso[:, b : b + 1],
                        scalar2=None, op0=mybir.AluOpType.mult,
                    )

            # =============== shared dense-aggregation pass =================
            def dense_agg(tab_sb):
                """aggT[f, d] accumulated over all 80 blocks; returns the
                3 PSUM group tiles (live until epilogue reads them)."""
                accs = [
                    pacc.tile(
                        [P, 512], F32, tag=f"acc{g}", space="PSUM",
                        name=f"acc{g}",
                    )
                    for g in range(3)
                ]
                for ac0 in range(0, NB_BLK, AC):
                    a8s = a8p.tile([P, AC, NPC], FP8, tag="a8s")
                    nc.sync.dma_start(
                        out=a8s[:],
                        in_=a8_in[:, ac0 * NPC : (ac0 + AC) * NPC].rearrange(
                            "p (b d) -> p b d", d=NPC
                        ),
                    )
                    for bi in range(AC):
                        b = ac0 + bi
                        for g, (g0, g1) in enumerate(GSPLIT):
                            nc.tensor.matmul(
                                out=accs[g][:, : g1 - g0],
                                lhsT=tab_sb[:, b, :],
                                rhs=a8s[:, bi, g0:g1],
                                start=(b == 0), stop=(b == NB_BLK - 1),
                                skip_group_check=True,
                            )
                return accs

            # =============== GC1 agg + GC2 node ===========================
            accs = dense_agg(h1)
            x2s = wp.tile([P, NPC], BF16, tag="x2s")
            for g, (g0, g1) in enumerate(GSPLIT):
                gw = g1 - g0
                t1 = wp.tile([P, 512], F32, tag="epi1")
                nc.vector.tensor_tensor(
                    out=t1[:, :gw], in0=accs[g][:, :gw], in1=ddbc[:, g0:g1],
                    op=mybir.AluOpType.mult,
                )
                nc.scalar.activation(
                    out=t1[:, :gw], in_=t1[:, :gw],
                    func=mybir.ActivationFunctionType.Relu,
                    bias=b1c[:, 0:1], scale=1.0,
                )
                nc.vector.tensor_tensor(
                    out=x2s[:, g0:g1], in0=t1[:, :gw], in1=dsbc[:, g0:g1],
                    op=mybir.AluOpType.mult,
                )
            # h2T[f2, d] = W2^T @ x2s
            h2t = wp.tile([P, NPC], F32, tag="h2t")
            for g, (g0, g1) in enumerate(GSPLIT):
                ps = pw.tile([P, 512], F32, tag="mmw", space="PSUM")
                nc.tensor.matmul(
                    out=ps[:, : g1 - g0], lhsT=w2[:], rhs=x2s[:, g0:g1],
                    start=True, stop=True,
                )
                nc.vector.tensor_copy(out=h2t[:, g0:g1], in_=ps[:, : g1 - g0])
            # transpose h2T tiles -> row-major h2 shard -> AG2 buffer
            for t in range(NTILES):
                w = TILE_W[t]
                pt = pw.tile([P, 512], F32, tag="mmw", space="PSUM")
                nc.tensor.transpose(
                    out=pt[:w, 0:P], in_=h2t[:, t * P : t * P + w],
                    identity=ident[:],
                )
                h2r = wp.tile([P, HID], BF16, tag="h2r")
                nc.vector.tensor_copy(out=h2r[:w, :], in_=pt[:w, 0:P])
                nc.sync.dma_start(
                    out=ag2_in[t * P : t * P + w, :], in_=h2r[:w, :]
                )

            nc.gpsimd.collective_compute(
                "AllGather", mybir.AluOpType.bypass, replica_groups=rg,
                ins=[ag2_in[:].opt()], outs=[tab2_d[:].opt()],
            )

            # =============== GC2 agg + GAT node prep ======================
            tab2 = tabp.tile([P, NB_BLK, HID], BF16, tag="tab")
            nc.sync.dma_start(
                out=tab2[:], in_=tab2_d[:].rearrange("(b p) f -> p b f", p=P)
            )
            accs = dense_agg(tab2)
            # x3T [f, d] bf16 (padded cols zeroed for the AG)
            x3t = cp.tile([P, NPAD], BF16, tag="x3t")
            nc.vector.memset(x3t[:, NPC:NPAD], 0.0)
            for g, (g0, g1) in enumerate(GSPLIT):
                gw = g1 - g0
                t1 = wp.tile([P, 512], F32, tag="epi1")
                nc.vector.tensor_tensor(
                    out=t1[:, :gw], in0=accs[g][:, :gw], in1=ddbc[:, g0:g1],
                    op=mybir.AluOpType.mult,
                )
                nc.scalar.activation(
                    out=x3t[:, g0:g1], in_=t1[:, :gw],
                    func=mybir.ActivationFunctionType.Relu,
                    bias=b2c[:, 0:1], scale=1.0,
                )
            nc.sync.dma_start(out=ag3_in[:], in_=x3t[:])
            # er per dst tile: [d, 4] = x3T_tile^T @ AR
            er_sb = cp.tile([P, NTILES * HEADS], BF16, tag="er_sb")
            nc.vector.memset(er_sb[:], 0.0)
            for t in range(NTILES):
                w = TILE_W[t]
                ps = psm.tile([P, 512], F32, tag="small", space="PSUM")
                nc.tensor.matmul(
                    out=ps[:w, 0:HEADS], lhsT=x3t[:, t * P : t * P + w],
                    rhs=ar4[:], start=True, stop=True,
                )
                nc.vector.tensor_copy(
                    out=er_sb[:w, t * HEADS : (t + 1) * HEADS],
                    in_=ps[:w, 0:HEADS],
                )

            nc.gpsimd.collective_compute(
                "AllGather", mybir.AluOpType.bypass, replica_groups=rg,
                ins=[ag3_in[:].opt()], outs=[tab3t_d[:].opt()],
            )

            # =============== ely table: [el|y] per node ===================
            tab3t = tabp.tile([P, NCORES, NTILES, P], BF16, tag="tab")
            nc.sync.dma_start(
                out=tab3t[:],
                in_=tab3t_d[:].rearrange(
                    "(c p) (j q) -> p c j q", p=P, q=P
                ),
            )
            ely = cp.tile([P, NB_BLK, 8], BF16, tag="ely")
            for g in range(2):
                ps = psm.tile([P, 512], F32, tag="small", space="PSUM")
                for bb in range(40):
                    b = g * 40 + bb
                    nc.tensor.matmul(
                        out=ps[:, bb * 8 : bb * 8 + 8],
                        lhsT=tab3t[:, b // NTILES, b % NTILES, :],
                        rhs=aly[:], start=True, stop=True,
                        skip_group_check=True,
                    )
                nc.vector.tensor_copy(
                    out=ely[:, g * 40 : (g + 1) * 40, :],
                    in_=ps[:, 0:320].rearrange("p (b f) -> p b f", f=8),
                )
            nc.sync.dma_start(
                out=ely_d[:, 0:8].rearrange("(b p) f -> p b f", p=P),
                in_=ely[:],
            )

            # =============== GAT edge phase ===============================
            for t in range(NTILES):
                w = TILE_W[t]
                gel = gelp.tile([P, nchunks, HID], BF16, tag="gel")
                qstep = -(-nchunks // 4)
                for c0 in range(0, nchunks, qstep):
                    c1 = min(c0 + qstep, nchunks)
                    nc.gpsimd.dma_gather(
                        gel[:, c0:c1, :], ely_d[:],
                        src16[:, t * IDXW + c0 * 8 : t * IDXW + c1 * 8],
                        (c1 - c0) * P, (c1 - c0) * P, HID, elem_step=HID,
                        single_packet=False, queue_num=next_q(),
                    )
                oh8 = ohp.tile([P, nchunks, P], FP8, tag="oh8")
                nc.scalar.dma_start(
                    out=oh8[:],
                    in_=oh8_in[
                        :, t * nchunks * P : (t + 1) * nchunks * P
                    ].rearrange("p (c d) -> p c d", d=P),
                )
                oht8 = ohtp.tile([P, nchunks, P], FP8, tag="oht8")
                nc.scalar.dma_start(
                    out=oht8[:],
                    in_=oht8_in[
                        :, t * nchunks * P : (t + 1) * nchunks * P
                    ].rearrange("p (c e) -> p c e", e=P),
                )
                acc_t = pga.tile([P, 512], F32, tag="gacc", space="PSUM")
                acc = acc_t[:, 0:8]
                ert = er_sb[:, t * HEADS : (t + 1) * HEADS]
                for b0 in range(0, nchunks, GB):
                    b1 = min(b0 + GB, nchunks)
                    nb = b1 - b0
                    erp_t = psm.tile([P, 512], F32, tag="small", space="PSUM")
                    erp = erp_t[:, 0 : GB * HEADS].rearrange(
                        "p (c h) -> p c h", h=HEADS
                    )
                    for cc in range(b0, b1):
                        nc.tensor.matmul(
                            out=erp[:, cc - b0, :], lhsT=oht8[:, cc, :],
                            rhs=ert, start=True, stop=True,
                            skip_group_check=True,
                        )
                    # e = lrelu(el + er); ex = exp(e)
                    e_all = wp.tile([P, GB, HEADS], F32, tag="e_all")
                    nc.vector.tensor_tensor(
                        out=e_all[:, :nb, :], in0=gel[:, b0:b1, 0:HEADS],
                        in1=erp[:, :nb, :], op=mybir.AluOpType.add,
                    )
                    nc.vector.scalar_tensor_tensor(
                        out=e_all[:, :nb, :], in0=e_all[:, :nb, :], scalar=0.2,
                        in1=e_all[:, :nb, :], op0=mybir.AluOpType.mult,
                        op1=mybir.AluOpType.max,
                    )
                    # rhs = [y*ex | ex] bf16; exp lands in rp directly
                    rp = wp.tile([P, GB, 8], BF16, tag="rp")
                    nc.scalar.activation(
                        out=rp[:, :nb, HEADS:8], in_=e_all[:, :nb, :],
                        func=mybir.ActivationFunctionType.Exp,
                    )
                    nc.vector.tensor_tensor(
                        out=rp[:, :nb, 0:HEADS], in0=gel[:, b0:b1, HEADS:8],
                        in1=rp[:, :nb, HEADS:8], op=mybir.AluOpType.mult,
                    )
                    for cc in range(b0, b1):
                        nc.tensor.matmul(
                            out=acc, lhsT=oh8[:, cc, :],
                            rhs=rp[:, cc - b0, :],
                            start=(cc == 0), stop=(cc == nchunks - 1),
                            skip_group_check=True,
                        )
                # epilogue: s = dsrc * (mean_h(yagg/den) + bgW3)
                den = wp.tile([P, HEADS], F32, tag="den")
                nc.vector.tensor_scalar(
                    out=den[:], in0=acc_t[:, HEADS:8], scalar1=1e-30,
                    scalar2=None, op0=mybir.AluOpType.max,
                )
                nc.vector.reciprocal(out=den[:], in_=den[:])
                wy = wp.tile([P, HEADS], F32, tag="wy")
                nc.vector.tensor_tensor(
                    out=wy[:], in0=acc_t[:, 0:HEADS], in1=den[:],
                    op=mybir.AluOpType.mult,
                )
                sv = wp.tile([P, 1], F32, tag="sv")
                nc.vector.reduce_sum(out=sv[:], in_=wy[:], axis=mybir.AxisListType.X)
                nc.vector.scalar_tensor_tensor(
                    out=sv[:], in0=sv[:], scalar=0.25, in1=bgw3[:],
                    op0=mybir.AluOpType.mult, op1=mybir.AluOpType.add,
                )
                svb = wp.tile([P, 1], BF16, tag="svb")
                nc.vector.tensor_scalar(
                    out=svb[:], in0=sv[:], scalar1=dscol[:, t : t + 1],
                    scalar2=None, op0=mybir.AluOpType.mult,
                )
                nc.sync.dma_start(
                    out=ag4_in[t * P : t * P + w, :], in_=svb[:w, :]
                )

            # prefetch the first two GC3 adjacency chunks during AG4
            a8pre = []
            for pc in range(2):
                a8s = a8p.tile([P, AC, NPC], FP8, tag="a8s", name=f"a8pre{pc}")
                nc.scalar.dma_start(
                    out=a8s[:],
                    in_=a8_in[:, pc * AC * NPC : (pc + 1) * AC * NPC].rearrange(
                        "p (b d) -> p b d", d=NPC
                    ),
                )
                a8pre.append(a8s)

            nc.gpsimd.collective_compute(
                "AllGather", mybir.AluOpType.bypass, replica_groups=rg,
                ins=[ag4_in[:].opt()], outs=[tabs_d[:].opt()],
            )

            # =============== GC3: dense matvec + sigmoid ==================
            s_sb = cp.tile([P, NB_BLK], BF16, tag="s_sb")
            nc.sync.dma_start(
                out=s_sb[:], in_=tabs_d[:].rearrange("(b p) one -> p (b one)", p=P)
            )
            acc3 = [
                pacc.tile(
                    [P, 512], F32, tag=f"acc{g}", space="PSUM", name=f"acc3{g}"
                )
                for g in range(3)
            ]
            for ac0 in range(0, NB_BLK, AC):
                ci = ac0 // AC
                if ci < 2:
                    a8s = a8pre[ci]
                else:
                    a8s = a8p.tile([P, AC, NPC], FP8, tag="a8s")
                    nc.sync.dma_start(
                        out=a8s[:],
                        in_=a8_in[:, ac0 * NPC : (ac0 + AC) * NPC].rearrange(
                            "p (b d) -> p b d", d=NPC
                        ),
                    )
                for bi in range(AC):
                    b = ac0 + bi
                    for g, (g0, g1) in enumerate(GSPLIT):
                        nc.tensor.matmul(
                            out=acc3[g][0:1, : g1 - g0],
                            lhsT=s_sb[:, b : b + 1],
                            rhs=a8s[:, bi, g0:g1],
                            start=(b == 0), stop=(b == NB_BLK - 1),
                            skip_group_check=True,
                        )
            risk_sb = wp.tile([P, NPC], F32, tag="risk")
            for g, (g0, g1) in enumerate(GSPLIT):
                gw = g1 - g0
                nc.vector.tensor_tensor(
                    out=risk_sb[0:1, g0:g1], in0=acc3[g][0:1, :gw],
                    in1=ddbc[0:1, g0:g1], op=mybir.AluOpType.mult,
                )
            nc.scalar.activation(
                out=risk_sb[0:1, :], in_=risk_sb[0:1, :],
                func=mybir.ActivationFunctionType.Sigmoid,
                bias=b3c[0:1, 0:1], scale=1.0,
            )
            nc.sync.dma_start(
                out=risk_out[:].rearrange("d one -> one d"),
                in_=risk_sb[0:1, :],
            )

    nc.compile()
    return nc


# ----------------------------------------------------------------------------
# host driver
# ----------------------------------------------------------------------------

def _get_program(nchunks):
    if nchunks not in _compiled_cache:
        _compiled_cache[nchunks] = _build(nchunks)
    return _compiled_cache[nchunks]


def _install_ntff_hook():
    """Profiling support: register the NTFF hook bass_utils expects when this
    image's antenv package lacks axon_hooks. Best-effort, trace-path only."""
    import types

    try:
        import antenv.axon_hooks  # noqa: F401

        return
    except ImportError:
        pass
    try:
        import antenv
        from trn_agent_boot.trn_boot import _ntff_profile_via_ctypes

        hook = _ntff_profile_via_ctypes("/opt/axon/libaxon_pjrt.so")
        mod = types.ModuleType("antenv.axon_hooks")
        mod.get_axon_ntff_profile_hook = lambda: hook
        mod.set_axon_ntff_profile_hook = lambda h: None
        sys.modules["antenv.axon_hooks"] = mod
        antenv.axon_hooks = mod
    except Exception:
        pass


def kernel(
    features, src, dst, W1, b1, W2, b2, W3, b3, Wg, attn_l, attn_r, bg,
    _trace=False,
):
    features = np.asarray(features, np.float32)
    per_core, deg, nchunks = _preprocess(src, dst)
    nc = _get_program(nchunks)

    # full features, wrapped + padded per 128-row block:
    # xtw[p, b, k, q] = x[row b*128+q, k*128+p]
    xpad = np.zeros((NFULL, IN_F), np.float32)
    for c in range(NCORES):
        xpad[c * NPAD : c * NPAD + NPC] = features[c * NPC : (c + 1) * NPC]
    xtw = (
        xpad.reshape(NB_BLK, P, 2, P)        # [b, q, k, p]
        .transpose(3, 0, 2, 1)               # [p, b, k, q]
        .reshape(P, 2 * NFULL)
        .astype(NP_BF16)
    )

    W1 = np.asarray(W1, np.float32)
    w1w = np.concatenate([W1[:P, :], W1[P:, :]], axis=1).astype(NP_BF16)
    Wg = np.asarray(Wg, np.float32)
    wghT = np.zeros((P, HEADS * HID), np.float32)
    for h in range(HEADS):
        wghT[:, h * HID : (h + 1) * HID] = Wg[:, h * HID : (h + 1) * HID].T

    common = dict(
        xtw=xtw,
        w1w=w1w,
        w2=np.asarray(W2, np.float32).astype(NP_BF16),
        wghT=wghT,
        alT=np.asarray(attn_l, np.float32).T.copy(),
        arT=np.asarray(attn_r, np.float32).T.copy(),
        w3c=np.asarray(W3, np.float32).reshape(P, 1),
        w3bc=np.tile(np.asarray(W3, np.float32).reshape(1, -1), (P, 1)),
        bgbc=np.tile(np.asarray(bg, np.float32).reshape(1, -1), (P, 1)),
        b1c=np.asarray(b1, np.float32).reshape(P, 1),
        b2c=np.asarray(b2, np.float32).reshape(P, 1),
        b3c=np.full((P, 1), np.float32(np.asarray(b3).reshape(-1)[0])),
    )
    in_maps = []
    for c in range(NCORES):
        m = dict(common)
        m["a8"] = per_core[c]["a8"]
        m["oh8"] = per_core[c]["oh8"]
        m["oht8"] = per_core[c]["ohT8"]
        m["src16"] = per_core[c]["src16"]
        m["doutblk"] = deg[c]["doutblk"]
        m["dinbc"] = deg[c]["dinbc"]
        m["doutownbc"] = deg[c]["doutownbc"]
        in_maps.append(m)

    if _trace:
        _install_ntff_hook()
    res = bass_utils.run_bass_kernel_spmd(
        nc, in_maps, core_ids=list(range(NCORES)), trace=_trace
    )
    out = np.concatenate([res.results[c]["risk"] for c in range(NCORES)], axis=0)
    if _trace:
        kernel.last_exec_time_ns = res.exec_time_ns
        kernel.last_results = res
    return out.astype(np.float32)


# revision 16
# speedup vs baseline: 1.2506x; 1.0696x over previous
"""Trainium2 Bass kernel for a 4-layer dependency GNN (3x GraphConv + GAT).

Full inputs in, full output out. Internally nodes are sharded across 8
NeuronCores by dst ownership (1250 nodes/core, padded to 1280 = 10 blocks
of 128 per shard).

Design (v2 — dense-adjacency):
  - The per-core adjacency A_c [10000 src x 1250 dst] (entry = edge
    multiplicity) is built on the host from the integer edge list and
    uploaded as fp8 (0/1/2 are exact).  GraphConv aggregations run as dense
    matmuls: aggT[f, d] = sum_b h_blk[b]^T @ A_blk[b]  (lhsT = 128-row
    table block, rhs = fp8 A block streamed from DRAM).  No per-edge
    gather and no one-hot builds for any GraphConv layer.
  - GC1 is fully replicated: every core receives the full feature matrix
    (bf16) and computes the full h1 table locally -> no AllGather before
    the first aggregation.
  - GAT: the GAT output is only consumed through mean_h(out)@W3, which is
    linear, so W3 is folded through the attention: per edge only
    y_h = x3 @ (Wg_h @ W3) (4 scalars) and ex_h (4 scalars) are
    aggregated.  Per-node [el|y] rows (16 B) are computed densely and
    per-edge rows fetched with one dma_gather per dst tile; the dst-side
    er term is expanded with transposed one-hot matmuls.  One-hot matrices
    (0/1) are uploaded from the host in fp8 and streamed.
  - Per-layer cross-core tables move through 4 small AllGathers
    (320 KB x 3 + 2.5 KB).

Host-side work is limited to index manipulation (edge bucketing, one-hot /
adjacency construction from integer indices, wrapping/padding, integer
degree counts) and dtype casts; all floating-point model math runs on
device (degree^-1/2 included).
"""

import sys

import numpy as np

sys.path.insert(0, "/opt/trn_rl_repo")

import ml_dtypes  # noqa: E402

import concourse.bacc as bacc  # noqa: E402
import concourse.mybir as mybir  # noqa: E402
import concourse.tile as tile  # noqa: E402
from concourse import bass_utils  # noqa: E402
from concourse.masks import make_identity  # noqa: E402

N = 10000
E = 320000
IN_F = 256
HID = 128
HEADS = 4
NCORES = 8
NPC = N // NCORES           # nodes per core (1250)
P = 128
NTILES = (NPC + P - 1) // P  # dst tiles per core (10)
TILE_W = [min(P, NPC - t * P) for t in range(NTILES)]
NPAD = NTILES * P            # padded shard rows (1280)
NB_BLK = NCORES * NTILES     # global 128-row src blocks (80)
NFULL = NCORES * NPAD        # padded table rows (10240)
GB = 7                       # chunk batch in the GAT edge phase
NQ = 4                       # SWDGE queues

F32 = mybir.dt.float32
BF16 = mybir.dt.bfloat16
FP8 = mybir.dt.float8e4
I16 = mybir.dt.int16

NP_BF16 = ml_dtypes.bfloat16
NP_FP8 = ml_dtypes.float8_e4m3

_compiled_cache = {}


# ----------------------------------------------------------------------------
# host-side sharding / index preprocessing (integer work + dtype casts only)
# ----------------------------------------------------------------------------

def _wrap16(idx_block):
    """dma_gather index layout: [16, n/16] with [p, s] = idx[s*16+p],
    replicated across the 8 gpsimd cores (8 groups of 16 partitions)."""
    n = idx_block.shape[0]
    assert n % 16 == 0
    base = idx_block.reshape(n // 16, 16).T.astype(np.int16)
    return np.tile(base, (8, 1))


def _prow(n):
    """Row of global node n in the 10240-row padded table."""
    return NPAD * (n // NPC) + (n % NPC)


# Block processing order: first blocks j<5 of every core (the "a" half of
# each AllGather shard), then j>=5 — lets the second half-collective overlap
# the first 40 blocks' matmuls.  PERM[i] = natural block id (10*c + j).
PERM = [10 * c + j for c in range(NCORES) for j in range(5)] + [
    10 * c + j for c in range(NCORES) for j in range(5, 10)
]


def _preprocess(src, dst):
    src = np.asarray(src).astype(np.int64).ravel()
    dst = np.asarray(dst).astype(np.int64).ravel()

    deg_out = np.bincount(src, minlength=N).astype(np.float32)
    deg_in = np.bincount(dst, minlength=N).astype(np.float32)

    # --- bucket edges by (dst core, dst tile) --------------------------------
    groups = {}
    counts = np.zeros((NCORES, NTILES), np.int64)
    for c in range(NCORES):
        sel = (dst // NPC) == c
        s_c = src[sel]
        d_c = dst[sel] - c * NPC
        order = np.argsort(d_c, kind="stable")
        s_c, d_c = s_c[order], d_c[order]
        t_c = d_c // P
        for t in range(NTILES):
            m = t_c == t
            groups[(c, t)] = (s_c[m], d_c[m] - t * P)
            counts[c, t] = int(m.sum())
    nchunks = int(-(-counts.max() // P))

    per_core = []
    for c in range(NCORES):
        # adjacency: [128, NB_BLK * NPC] fp8; block b covers global src rows
        # [NPC*(b//NTILES) + P*(b%NTILES), +128) (rows beyond the 98-wide
        # tail blocks stay zero).
        a = np.zeros((P, NB_BLK * NPC), np.float32)
        # one-hots for the GAT edge phase: [128, NTILES*nchunks*128]
        oh = np.zeros((P, NTILES * nchunks * P), np.float32)
        ohT = np.zeros((P, NTILES * nchunks * P), np.float32)
        idx_blocks = []
        for t in range(NTILES):
            s_g, dl_g = groups[(c, t)]
            ne = len(s_g)
            # adjacency entries for this tile's edges
            sl = s_g % NPC
            blk = NTILES * (s_g // NPC) + sl // P
            srow = sl % P
            np.add.at(a, (srow, blk * NPC + t * P + dl_g), 1.0)
            # per-edge slots: edge i -> chunk i//128, lane i%128
            ch = np.arange(ne) // P
            lane = np.arange(ne) % P
            base = (t * nchunks + ch) * P
            oh[lane, base + dl_g] = 1.0
            ohT[dl_g, base + lane] = 1.0
            # gather indices (padded rows use index 0 -> finite garbage,
            # masked by zero one-hot columns)
            idx = np.zeros(nchunks * P, np.int64)
            idx[:ne] = _prow(s_g)
            idx_blocks.append(_wrap16(idx))
        a = a.reshape(P, NB_BLK, NPC)[:, PERM, :].reshape(P, NB_BLK * NPC)
        per_core.append(
            dict(
                a8=np.ascontiguousarray(a).astype(NP_FP8),
                oh8=oh.astype(NP_FP8),
                ohT8=ohT.astype(NP_FP8),
                src16=np.concatenate(idx_blocks, axis=1),
            )
        )

    # --- degree tensors (raw counts; device computes clip+rsqrt) -------------
    deg = []
    for c in range(NCORES):
        own_out = deg_out[c * NPC : (c + 1) * NPC]
        own_in = deg_in[c * NPC : (c + 1) * NPC]
        dout_blk = np.zeros((P, NB_BLK), np.float32)
        for i, b in enumerate(PERM):
            g0 = NPC * (b // NTILES) + P * (b % NTILES)
            w = min(P, NPC * (b // NTILES) + NPC - g0)
            dout_blk[:w, i] = deg_out[g0 : g0 + w]
        deg.append(
            dict(
                doutblk=dout_blk,
                dinbc=np.tile(own_in[None, :], (P, 1)),
                doutownbc=np.tile(own_out[None, :], (P, 1)),
            )
        )
    return per_core, deg, nchunks


# ----------------------------------------------------------------------------
# device program
# ----------------------------------------------------------------------------

def _build(nchunks):
    perm = PERM
    IDXW = nchunks * 8           # int16 idx cols per tile
    NBATCH = -(-nchunks // GB)   # GAT chunk batches per tile
    AC = 10                      # adjacency blocks per DMA chunk
    GSPLIT = [(0, 512), (512, 1024), (1024, NPC)]  # dst column groups

    nc = bacc.Bacc(
        "TRN2", target_bir_lowering=False, debug=False, num_devices=NCORES,
        num_swdge_queues=NQ,
    )

    _q = [0]

    def next_q():
        _q[0] = (_q[0] + 1) % NQ
        return _q[0]

    # --- I/O ----------------------------------------------------------------
    xtw_in = nc.dram_tensor("xtw", [P, 2 * NFULL], BF16, kind="ExternalInput")
    a8_in = nc.dram_tensor("a8", [P, NB_BLK * NPC], FP8, kind="ExternalInput")
    oh8_in = nc.dram_tensor(
        "oh8", [P, NTILES * nchunks * P], FP8, kind="ExternalInput"
    )
    oht8_in = nc.dram_tensor(
        "oht8", [P, NTILES * nchunks * P], FP8, kind="ExternalInput"
    )
    src16_in = nc.dram_tensor("src16", [P, NTILES * IDXW], I16, kind="ExternalInput")
    w1w_in = nc.dram_tensor("w1w", [P, 2 * HID], BF16, kind="ExternalInput")
    w2_in = nc.dram_tensor("w2", [P, HID], BF16, kind="ExternalInput")
    wghT_in = nc.dram_tensor("wghT", [P, HEADS * HID], F32, kind="ExternalInput")
    alT_in = nc.dram_tensor("alT", [P, HEADS], F32, kind="ExternalInput")
    arT_in = nc.dram_tensor("arT", [P, HEADS], F32, kind="ExternalInput")
    w3c_in = nc.dram_tensor("w3c", [P, 1], F32, kind="ExternalInput")
    w3bc_in = nc.dram_tensor("w3bc", [P, HID], F32, kind="ExternalInput")
    bgbc_in = nc.dram_tensor("bgbc", [P, HEADS * HID], F32, kind="ExternalInput")
    b1c_in = nc.dram_tensor("b1c", [P, 1], F32, kind="ExternalInput")
    b2c_in = nc.dram_tensor("b2c", [P, 1], F32, kind="ExternalInput")
    b3c_in = nc.dram_tensor("b3c", [P, 1], F32, kind="ExternalInput")
    doutblk_in = nc.dram_tensor("doutblk", [P, NB_BLK], F32, kind="ExternalInput")
    dinbc_in = nc.dram_tensor("dinbc", [P, NPC], F32, kind="ExternalInput")
    doutownbc_in = nc.dram_tensor("doutownbc", [P, NPC], F32, kind="ExternalInput")
    risk_out = nc.dram_tensor("risk", [NPC, 1], F32, kind="ExternalOutput")

    rg = [list(range(NCORES))]

    with tile.TileContext(nc) as tc:
        with (
            tc.tile_pool(name="const", bufs=1) as cp,
            tc.tile_pool(name="tab", bufs=1) as tabp,
            tc.tile_pool(name="a8", bufs=3) as a8p,
            tc.tile_pool(name="xs", bufs=2) as xp,
            tc.tile_pool(name="oh", bufs=2) as ohp,
            tc.tile_pool(name="oht", bufs=2) as ohtp,
            tc.tile_pool(name="gel", bufs=4) as gelp,
            tc.tile_pool(name="work", bufs=3) as wp,
            tc.tile_pool(name="acc", bufs=1, space="PSUM") as pacc,
            tc.tile_pool(name="pmm", bufs=2, space="PSUM") as pw,
            tc.tile_pool(name="pga", bufs=1, space="PSUM") as pga,
            tc.tile_pool(name="psm", bufs=2, space="PSUM") as psm,
            tc.tile_pool(name="dram", bufs=1, space="DRAM") as dram,
        ):
            # --- DRAM interchange buffers ---------------------------------
            ag2_in = dram.tile([NPAD, HID], BF16)
            tab2a_d = dram.tile([NCORES * 5 * P, HID], BF16, addr_space="Shared")
            tab2b_d = dram.tile([NCORES * 5 * P, HID], BF16, addr_space="Shared")
            ag3_in = dram.tile([P, NPAD], BF16)
            tab3t_d = dram.tile([NCORES * P, NPAD], BF16, addr_space="Shared")
            ely_d = dram.tile([NFULL, HID], BF16)  # payload in cols 0:8
            ag4_in = dram.tile([NPAD, 1], BF16)
            tabs_d = dram.tile([NFULL, 1], BF16, addr_space="Shared")

            # --- resident constants ---------------------------------------
            def cload(name, dram_t, shape, dt):
                t = cp.tile(shape, dt, tag=name)
                nc.sync.dma_start(out=t[:], in_=dram_t[:])
                return t

            w1w = cp.tile([P, 2, HID], BF16, tag="w1w")
            nc.sync.dma_start(
                out=w1w[:], in_=w1w_in[:].rearrange("p (k f) -> p k f", k=2)
            )
            w2 = cload("w2", w2_in, [P, HID], BF16)
            wghT = cload("wghT", wghT_in, [P, HEADS * HID], F32)
            alT = cload("alT", alT_in, [P, HEADS], F32)
            arT = cload("arT", arT_in, [P, HEADS], F32)
            w3c = cload("w3c", w3c_in, [P, 1], F32)
            w3bc = cload("w3bc", w3bc_in, [P, HID], F32)
            bgbc = cload("bgbc", bgbc_in, [P, HEADS * HID], F32)
            b1c = cload("b1c", b1c_in, [P, 1], F32)
            b2c = cload("b2c", b2c_in, [P, 1], F32)
            b3c = cload("b3c", b3c_in, [P, 1], F32)
            src16 = cload("src16", src16_in, [P, NTILES * IDXW], I16)

            ident = cp.tile([P, P], F32)
            make_identity(nc, ident[:])

            zrow = cp.tile([P, HID], BF16, tag="zrow")
            nc.vector.memset(zrow[:], 0.0)

            def rsqrt_inplace(t):
                nc.vector.tensor_scalar(
                    out=t[:], in0=t[:], scalar1=1.0, scalar2=None,
                    op0=mybir.AluOpType.max,
                )
                nc.vector.reciprocal(out=t[:], in_=t[:])
                nc.scalar.activation(
                    out=t[:], in_=t[:], func=mybir.ActivationFunctionType.Sqrt
                )
                return t

            dso = rsqrt_inplace(cload("doutblk", doutblk_in, [P, NB_BLK], F32))
            ddbc = rsqrt_inplace(cload("dinbc", dinbc_in, [P, NPC], F32))
            dsbc = rsqrt_inplace(cload("doutownbc", doutownbc_in, [P, NPC], F32))

            # dsrc for own nodes as per-tile columns: dscol[p, t] =
            # dsrc[t*128+p], read off the broadcast dsbc rows via a diagonal
            # mask + free-dim reduce.
            dscol = cp.tile([P, NTILES], F32, tag="dscol")
            for t in range(NTILES):
                w = TILE_W[t]
                tmp = wp.tile([P, P], F32, tag="diag")
                nc.vector.tensor_tensor(
                    out=tmp[:w, :w], in0=dsbc[:w, t * P : t * P + w],
                    in1=ident[:w, :w], op=mybir.AluOpType.mult,
                )
                nc.vector.reduce_sum(
                    out=dscol[:w, t : t + 1], in_=tmp[:w, :w],
                    axis=mybir.AxisListType.X,
                )

            # bgW3 = (mean_h bg_h) @ W3 as a [128, 1] broadcast column
            bgm = cp.tile([P, HID], F32, tag="bgm")
            nc.vector.tensor_tensor(
                out=bgm[:], in0=bgbc[:, 0:HID], in1=bgbc[:, HID : 2 * HID],
                op=mybir.AluOpType.add,
            )
            nc.vector.tensor_tensor(
                out=bgm[:], in0=bgm[:], in1=bgbc[:, 2 * HID : 3 * HID],
                op=mybir.AluOpType.add,
            )
            nc.vector.tensor_tensor(
                out=bgm[:], in0=bgm[:], in1=bgbc[:, 3 * HID : 4 * HID],
                op=mybir.AluOpType.add,
            )
            nc.vector.tensor_scalar(
                out=bgm[:], in0=bgm[:], scalar1=0.25, scalar2=None,
                op0=mybir.AluOpType.mult,
            )
            bgw3 = cp.tile([P, 1], F32, tag="bgw3")
            nc.vector.tensor_tensor(
                out=bgm[:], in0=bgm[:], in1=w3bc[:], op=mybir.AluOpType.mult
            )
            nc.vector.reduce_sum(out=bgw3[:], in_=bgm[:], axis=mybir.AxisListType.X)

            # ALY [128, 8] = [AL | AY], AR [128, 4]: per head h,
            # col = Wg_h^T-matmul with [alT_h | arT_h | W3]
            alyr_t = psm.tile([P, 512], F32, tag="small", space="PSUM")
            alyr_ps = alyr_t[:, 0:3]
            aly = cp.tile([P, 2 * HEADS], BF16, tag="aly")
            ar4 = cp.tile([P, HEADS], BF16, tag="ar4")
            for h in range(HEADS):
                rhs3 = wp.tile([P, 3], F32, tag="rhs3")
                nc.vector.tensor_copy(out=rhs3[:, 0:1], in_=alT[:, h : h + 1])
                nc.vector.tensor_copy(out=rhs3[:, 1:2], in_=arT[:, h : h + 1])
                nc.vector.tensor_copy(out=rhs3[:, 2:3], in_=w3c[:, 0:1])
                nc.tensor.matmul(
                    out=alyr_ps, lhsT=wghT[:, h * HID : (h + 1) * HID],
                    rhs=rhs3[:], start=True, stop=True,
                )
                nc.vector.tensor_copy(out=aly[:, h : h + 1], in_=alyr_t[:, 0:1])
                nc.vector.tensor_copy(out=ar4[:, h : h + 1], in_=alyr_t[:, 1:2])
                nc.vector.tensor_copy(
                    out=aly[:, HEADS + h : HEADS + h + 1], in_=alyr_t[:, 2:3]
                )

            # zero the pad rows of the DRAM AG buffers once
            nc.sync.dma_start(out=ag2_in[NPC:NPAD, :], in_=zrow[: NPAD - NPC, :])
            nc.sync.dma_start(
                out=ag4_in[NPC:NPAD, :], in_=zrow[: NPAD - NPC, 0:1]
            )

            # =============== GC1: replicated node transform ================
            # h1_full[n, f] = dsrc[n] * (x[n] @ W1), all 10240 padded rows.
            h1 = tabp.tile([P, NB_BLK, HID], BF16, tag="tab")
            XC = 10  # blocks per xT stream chunk
            for cc0 in range(0, NB_BLK, XC):
                xs = xp.tile([P, XC, 2, P], BF16, tag="xs")
                nc.sync.dma_start(
                    out=xs[:],
                    in_=xtw_in[
                        :, cc0 * 2 * P : (cc0 + XC) * 2 * P
                    ].rearrange("p (b k q) -> p b k q", k=2, q=P),
                )
                for bi in range(XC):
                    b = cc0 + bi
                    ps = pw.tile([P, 512], F32, tag="mmw", space="PSUM")
                    nc.tensor.matmul(
                        out=ps[:, 0:HID], lhsT=xs[:, bi, 0, :],
                        rhs=w1w[:, 0, :], start=True, stop=False,
                    )
                    nc.tensor.matmul(
                        out=ps[:, 0:HID], lhsT=xs[:, bi, 1, :],
                        rhs=w1w[:, 1, :], start=False, stop=True,
                    )
                    nc.vector.tensor_scalar(
                        out=h1[:, b, :], in0=ps[:, 0:HID],
                        scalar1=dso[:, b : b + 1],
                        scalar2=None, op0=mybir.AluOpType.mult,
                    )

            # =============== shared dense-aggregation pass =================
            def dense_agg(tab_sb):
                """aggT[f, d] accumulated over all 80 blocks; returns the
                3 PSUM group tiles (live until epilogue reads them)."""
                accs = [
                    pacc.tile(
                        [P, 512], F32, tag=f"acc{g}", space="PSUM",
                        name=f"acc{g}",
                    )
                    for g in range(3)
                ]
                for ac0 in range(0, NB_BLK, AC):
                    a8s = a8p.tile([P, AC, NPC], FP8, tag="a8s")
                    nc.scalar.dma_start(
                        out=a8s[:],
                        in_=a8_in[:, ac0 * NPC : (ac0 + AC) * NPC].rearrange(
                            "p (b d) -> p b d", d=NPC
                        ),
                    )
                    for bi in range(AC):
                        b = ac0 + bi
                        for g, (g0, g1) in enumerate(GSPLIT):
                            nc.tensor.matmul(
                                out=accs[g][:, : g1 - g0],
                                lhsT=tab_sb[:, b, :],
                                rhs=a8s[:, bi, g0:g1],
                                start=(b == 0), stop=(b == NB_BLK - 1),
                                skip_group_check=True,
                            )
                return accs

            # =============== GC1 agg + GC2 node ===========================
            accs = dense_agg(h1)
            x2s = wp.tile([P, NPC], BF16, tag="x2s")
            for g, (g0, g1) in enumerate(GSPLIT):
                gw = g1 - g0
                t1 = wp.tile([P, 512], F32, tag="epi1")
                nc.vector.tensor_tensor(
                    out=t1[:, :gw], in0=accs[g][:, :gw], in1=ddbc[:, g0:g1],
                    op=mybir.AluOpType.mult,
                )
                nc.scalar.activation(
                    out=t1[:, :gw], in_=t1[:, :gw],
                    func=mybir.ActivationFunctionType.Relu,
                    bias=b1c[:, 0:1], scale=1.0,
                )
                nc.vector.tensor_tensor(
                    out=x2s[:, g0:g1], in0=t1[:, :gw], in1=dsbc[:, g0:g1],
                    op=mybir.AluOpType.mult,
                )
            # h2T[f2, d] = W2^T @ x2s
            h2t = wp.tile([P, NPC], F32, tag="h2t")
            for g, (g0, g1) in enumerate(GSPLIT):
                ps = pw.tile([P, 512], F32, tag="mmw", space="PSUM")
                nc.tensor.matmul(
                    out=ps[:, : g1 - g0], lhsT=w2[:], rhs=x2s[:, g0:g1],
                    start=True, stop=True,
                )
                nc.vector.tensor_copy(out=h2t[:, g0:g1], in_=ps[:, : g1 - g0])
            # transpose h2T tiles -> row-major h2 shard -> AG2 buffer
            for t in range(NTILES):
                w = TILE_W[t]
                pt = pw.tile([P, 512], F32, tag="mmw", space="PSUM")
                nc.tensor.transpose(
                    out=pt[:w, 0:P], in_=h2t[:, t * P : t * P + w],
                    identity=ident[:],
                )
                h2r = wp.tile([P, HID], BF16, tag="h2r")
                nc.vector.tensor_copy(out=h2r[:w, :], in_=pt[:w, 0:P])
                nc.sync.dma_start(
                    out=ag2_in[t * P : t * P + w, :], in_=h2r[:w, :]
                )

            nc.gpsimd.collective_compute(
                "AllGather", mybir.AluOpType.bypass, replica_groups=rg,
                ins=[ag2_in[0 : 5 * P, :].opt()], outs=[tab2a_d[:].opt()],
            )
            nc.gpsimd.collective_compute(
                "AllGather", mybir.AluOpType.bypass, replica_groups=rg,
                ins=[ag2_in[5 * P : NPAD, :].opt()], outs=[tab2b_d[:].opt()],
            )

            # =============== GC2 agg + GAT node prep ======================
            tab2 = tabp.tile([P, NB_BLK, HID], BF16, tag="tab")
            nc.sync.dma_start(
                out=tab2[:, 0:40, :],
                in_=tab2a_d[:].rearrange("(b p) f -> p b f", p=P),
            )
            nc.sync.dma_start(
                out=tab2[:, 40:80, :],
                in_=tab2b_d[:].rearrange("(b p) f -> p b f", p=P),
            )
            accs = dense_agg(tab2)
            # x3T [f, d] bf16 (padded cols zeroed for the AG)
            x3t = cp.tile([P, NPAD], BF16, tag="x3t")
            nc.vector.memset(x3t[:, NPC:NPAD], 0.0)
            for g, (g0, g1) in enumerate(GSPLIT):
                gw = g1 - g0
                t1 = wp.tile([P, 512], F32, tag="epi1")
                nc.vector.tensor_tensor(
                    out=t1[:, :gw], in0=accs[g][:, :gw], in1=ddbc[:, g0:g1],
                    op=mybir.AluOpType.mult,
                )
                nc.scalar.activation(
                    out=x3t[:, g0:g1], in_=t1[:, :gw],
                    func=mybir.ActivationFunctionType.Relu,
                    bias=b2c[:, 0:1], scale=1.0,
                )
            nc.sync.dma_start(out=ag3_in[:], in_=x3t[:])
            # er per dst tile: [d, 4] = x3T_tile^T @ AR
            er_sb = cp.tile([P, NTILES * HEADS], BF16, tag="er_sb")
            nc.vector.memset(er_sb[:], 0.0)
            for t in range(NTILES):
                w = TILE_W[t]
                ps = psm.tile([P, 512], F32, tag="small", space="PSUM")
                nc.tensor.matmul(
                    out=ps[:w, 0:HEADS], lhsT=x3t[:, t * P : t * P + w],
                    rhs=ar4[:], start=True, stop=True,
                )
                nc.vector.tensor_copy(
                    out=er_sb[:w, t * HEADS : (t + 1) * HEADS],
                    in_=ps[:w, 0:HEADS],
                )

            nc.gpsimd.collective_compute(
                "AllGather", mybir.AluOpType.bypass, replica_groups=rg,
                ins=[ag3_in[:].opt()], outs=[tab3t_d[:].opt()],
            )

            # =============== ely table: [el|y] per node ===================
            tab3t = tabp.tile([P, NCORES, NTILES, P], BF16, tag="tab")
            nc.sync.dma_start(
                out=tab3t[:],
                in_=tab3t_d[:].rearrange(
                    "(c p) (j q) -> p c j q", p=P, q=P
                ),
            )
            ely = cp.tile([P, NB_BLK, 8], BF16, tag="ely")
            for g in range(2):
                ps = psm.tile([P, 512], F32, tag="small", space="PSUM")
                for bb in range(40):
                    b = g * 40 + bb
                    nc.tensor.matmul(
                        out=ps[:, bb * 8 : bb * 8 + 8],
                        lhsT=tab3t[:, b // NTILES, b % NTILES, :],
                        rhs=aly[:], start=True, stop=True,
                        skip_group_check=True,
                    )
                nc.vector.tensor_copy(
                    out=ely[:, g * 40 : (g + 1) * 40, :],
                    in_=ps[:, 0:320].rearrange("p (b f) -> p b f", f=8),
                )
            nc.sync.dma_start(
                out=ely_d[:, 0:8].rearrange("(b p) f -> p b f", p=P),
                in_=ely[:],
            )

            # =============== GAT edge phase ===============================
            for t in range(NTILES):
                w = TILE_W[t]
                gel = gelp.tile([P, nchunks, HID], BF16, tag="gel")
                qstep = -(-nchunks // 4)
                for c0 in range(0, nchunks, qstep):
                    c1 = min(c0 + qstep, nchunks)
                    nc.gpsimd.dma_gather(
                        gel[:, c0:c1, :], ely_d[:],
                        src16[:, t * IDXW + c0 * 8 : t * IDXW + c1 * 8],
                        (c1 - c0) * P, (c1 - c0) * P, HID, elem_step=HID,
                        single_packet=False, queue_num=next_q(),
                    )
                oh8 = ohp.tile([P, nchunks, P], FP8, tag="oh8")
                nc.scalar.dma_start(
                    out=oh8[:],
                    in_=oh8_in[
                        :, t * nchunks * P : (t + 1) * nchunks * P
                    ].rearrange("p (c d) -> p c d", d=P),
                )
                oht8 = ohtp.tile([P, nchunks, P], FP8, tag="oht8")
                nc.scalar.dma_start(
                    out=oht8[:],
                    in_=oht8_in[
                        :, t * nchunks * P : (t + 1) * nchunks * P
                    ].rearrange("p (c e) -> p c e", e=P),
                )
                acc_t = pga.tile([P, 512], F32, tag="gacc", space="PSUM")
                acc = acc_t[:, 0:8]
                ert = er_sb[:, t * HEADS : (t + 1) * HEADS]
                for b0 in range(0, nchunks, GB):
                    b1 = min(b0 + GB, nchunks)
                    nb = b1 - b0
                    erp_t = psm.tile([P, 512], F32, tag="small", space="PSUM")
                    erp = erp_t[:, 0 : GB * HEADS].rearrange(
                        "p (c h) -> p c h", h=HEADS
                    )
                    for cc in range(b0, b1):
                        nc.tensor.matmul(
                            out=erp[:, cc - b0, :], lhsT=oht8[:, cc, :],
                            rhs=ert, start=True, stop=True,
                            skip_group_check=True,
                        )
                    # e = lrelu(el + er); ex = exp(e)
                    e_all = wp.tile([P, GB, HEADS], F32, tag="e_all")
                    nc.vector.tensor_tensor(
                        out=e_all[:, :nb, :], in0=gel[:, b0:b1, 0:HEADS],
                        in1=erp[:, :nb, :], op=mybir.AluOpType.add,
                    )
                    nc.vector.scalar_tensor_tensor(
                        out=e_all[:, :nb, :], in0=e_all[:, :nb, :], scalar=0.2,
                        in1=e_all[:, :nb, :], op0=mybir.AluOpType.mult,
                        op1=mybir.AluOpType.max,
                    )
                    # rhs = [y*ex | ex] bf16; exp lands in rp directly
                    rp = wp.tile([P, GB, 8], BF16, tag="rp")
                    nc.scalar.activation(
                        out=rp[:, :nb, HEADS:8], in_=e_all[:, :nb, :],
                        func=mybir.ActivationFunctionType.Exp,
                    )
                    nc.vector.tensor_tensor(
                        out=rp[:, :nb, 0:HEADS], in0=gel[:, b0:b1, HEADS:8],
                        in1=rp[:, :nb, HEADS:8], op=mybir.AluOpType.mult,
                    )
                    for cc in range(b0, b1):
                        nc.tensor.matmul(
                            out=acc, lhsT=oh8[:, cc, :],
                            rhs=rp[:, cc - b0, :],
                            start=(cc == 0), stop=(cc == nchunks - 1),
                            skip_group_check=True,
                        )
                # epilogue: s = dsrc * (mean_h(yagg/den) + bgW3)
                den = wp.tile([P, HEADS], F32, tag="den")
                nc.vector.tensor_scalar(
                    out=den[:], in0=acc_t[:, HEADS:8], scalar1=1e-30,
                    scalar2=None, op0=mybir.AluOpType.max,
                )
                nc.vector.reciprocal(out=den[:], in_=den[:])
                wy = wp.tile([P, HEADS], F32, tag="wy")
                nc.vector.tensor_tensor(
                    out=wy[:], in0=acc_t[:, 0:HEADS], in1=den[:],
                    op=mybir.AluOpType.mult,
                )
                sv = wp.tile([P, 1], F32, tag="sv")
                nc.vector.reduce_sum(out=sv[:], in_=wy[:], axis=mybir.AxisListType.X)
                nc.vector.scalar_tensor_tensor(
                    out=sv[:], in0=sv[:], scalar=0.25, in1=bgw3[:],
                    op0=mybir.AluOpType.mult, op1=mybir.AluOpType.add,
                )
                svb = wp.tile([P, 1], BF16, tag="svb")
                nc.vector.tensor_scalar(
                    out=svb[:], in0=sv[:], scalar1=dscol[:, t : t + 1],
                    scalar2=None, op0=mybir.AluOpType.mult,
                )
                nc.sync.dma_start(
                    out=ag4_in[t * P : t * P + w, :], in_=svb[:w, :]
                )

            # prefetch the first two GC3 adjacency chunks during AG4
            a8pre = []
            for pc in range(3):
                a8s = a8p.tile([P, AC, NPC], FP8, tag="a8s", name=f"a8pre{pc}")
                nc.scalar.dma_start(
                    out=a8s[:],
                    in_=a8_in[:, pc * AC * NPC : (pc + 1) * AC * NPC].rearrange(
                        "p (b d) -> p b d", d=NPC
                    ),
                )
                a8pre.append(a8s)

            nc.gpsimd.collective_compute(
                "AllGather", mybir.AluOpType.bypass, replica_groups=rg,
                ins=[ag4_in[:].opt()], outs=[tabs_d[:].opt()],
            )

            # =============== GC3: dense matvec + sigmoid ==================
            s_sb = cp.tile([P, NB_BLK], BF16, tag="s_sb")
            nc.sync.dma_start(
                out=s_sb[:], in_=tabs_d[:].rearrange("(b p) one -> p (b one)", p=P)
            )
            acc3 = [
                pacc.tile(
                    [P, 512], F32, tag=f"acc{g}", space="PSUM", name=f"acc3{g}"
                )
                for g in range(3)
            ]
            for ac0 in range(0, NB_BLK, AC):
                ci = ac0 // AC
                if ci < 3:
                    a8s = a8pre[ci]
                else:
                    a8s = a8p.tile([P, AC, NPC], FP8, tag="a8s")
                    nc.scalar.dma_start(
                        out=a8s[:],
                        in_=a8_in[:, ac0 * NPC : (ac0 + AC) * NPC].rearrange(
                            "p (b d) -> p b d", d=NPC
                        ),
                    )
                for bi in range(AC):
                    b = ac0 + bi
                    sb = perm[b]
                    for g, (g0, g1) in enumerate(GSPLIT):
                        nc.tensor.matmul(
                            out=acc3[g][0:1, : g1 - g0],
                            lhsT=s_sb[:, sb : sb + 1],
                            rhs=a8s[:, bi, g0:g1],
                            start=(b == 0), stop=(b == NB_BLK - 1),
                            skip_group_check=True,
                        )
            risk_sb = wp.tile([P, NPC], F32, tag="risk")
            for g, (g0, g1) in enumerate(GSPLIT):
                gw = g1 - g0
                nc.vector.tensor_tensor(
                    out=risk_sb[0:1, g0:g1], in0=acc3[g][0:1, :gw],
                    in1=ddbc[0:1, g0:g1], op=mybir.AluOpType.mult,
                )
            nc.scalar.activation(
                out=risk_sb[0:1, :], in_=risk_sb[0:1, :],
                func=mybir.ActivationFunctionType.Sigmoid,
                bias=b3c[0:1, 0:1], scale=1.0,
            )
            nc.sync.dma_start(
                out=risk_out[:].rearrange("d one -> one d"),
                in_=risk_sb[0:1, :],
            )

    nc.compile()
    return nc


# ----------------------------------------------------------------------------
# host driver
# ----------------------------------------------------------------------------

def _get_program(nchunks):
    if nchunks not in _compiled_cache:
        _compiled_cache[nchunks] = _build(nchunks)
    return _compiled_cache[nchunks]


def _install_ntff_hook():
    """Profiling support: register the NTFF hook bass_utils expects when this
    image's antenv package lacks axon_hooks. Best-effort, trace-path only."""
    import types

    try:
        import antenv.axon_hooks  # noqa: F401

        return
    except ImportError:
        pass
    try:
        import antenv
        from trn_agent_boot.trn_boot import _ntff_profile_via_ctypes

        hook = _ntff_profile_via_ctypes("/opt/axon/libaxon_pjrt.so")
        mod = types.ModuleType("antenv.axon_hooks")
        mod.get_axon_ntff_profile_hook = lambda: hook
        mod.set_axon_ntff_profile_hook = lambda h: None
        sys.modules["antenv.axon_hooks"] = mod
        antenv.axon_hooks = mod
    except Exception:
        pass


def kernel(
    features, src, dst, W1, b1, W2, b2, W3, b3, Wg, attn_l, attn_r, bg,
    _trace=False,
):
    features = np.asarray(features, np.float32)
    per_core, deg, nchunks = _preprocess(src, dst)
    nc = _get_program(nchunks)

    # full features, wrapped + padded per 128-row block:
    # xtw[p, b, k, q] = x[row b*128+q, k*128+p]
    xpad = np.zeros((NFULL, IN_F), np.float32)
    for c in range(NCORES):
        xpad[c * NPAD : c * NPAD + NPC] = features[c * NPC : (c + 1) * NPC]
    xtw = (
        xpad.reshape(NB_BLK, P, 2, P)[PERM]  # [b, q, k, p] in PERM order
        .transpose(3, 0, 2, 1)               # [p, b, k, q]
        .reshape(P, 2 * NFULL)
        .astype(NP_BF16)
    )

    W1 = np.asarray(W1, np.float32)
    w1w = np.concatenate([W1[:P, :], W1[P:, :]], axis=1).astype(NP_BF16)
    Wg = np.asarray(Wg, np.float32)
    wghT = np.zeros((P, HEADS * HID), np.float32)
    for h in range(HEADS):
        wghT[:, h * HID : (h + 1) * HID] = Wg[:, h * HID : (h + 1) * HID].T

    common = dict(
        xtw=xtw,
        w1w=w1w,
        w2=np.asarray(W2, np.float32).astype(NP_BF16),
        wghT=wghT,
        alT=np.asarray(attn_l, np.float32).T.copy(),
        arT=np.asarray(attn_r, np.float32).T.copy(),
        w3c=np.asarray(W3, np.float32).reshape(P, 1),
        w3bc=np.tile(np.asarray(W3, np.float32).reshape(1, -1), (P, 1)),
        bgbc=np.tile(np.asarray(bg, np.float32).reshape(1, -1), (P, 1)),
        b1c=np.asarray(b1, np.float32).reshape(P, 1),
        b2c=np.asarray(b2, np.float32).reshape(P, 1),
        b3c=np.full((P, 1), np.float32(np.asarray(b3).reshape(-1)[0])),
    )
    in_maps = []
    for c in range(NCORES):
        m = dict(common)
        m["a8"] = per_core[c]["a8"]
        m["oh8"] = per_core[c]["oh8"]
        m["oht8"] = per_core[c]["ohT8"]
        m["src16"] = per_core[c]["src16"]
        m["doutblk"] = deg[c]["doutblk"]
        m["dinbc"] = deg[c]["dinbc"]
        m["doutownbc"] = deg[c]["doutownbc"]
        in_maps.append(m)

    if _trace:
        _install_ntff_hook()
    res = bass_utils.run_bass_kernel_spmd(
        nc, in_maps, core_ids=list(range(NCORES)), trace=_trace
    )
    out = np.concatenate([res.results[c]["risk"] for c in range(NCORES)], axis=0)
    if _trace:
        kernel.last_exec_time_ns = res.exec_time_ns
        kernel.last_results = res
    return out.astype(np.float32)
